# revision 16
# baseline (speedup 1.0000x reference)
"""nn_Attention Trainium2 Bass kernel (optimized).

Full attention forward: x->(q,k,v) with l2-normalized weights, per-head-dim
l2 norm + learned qk scale, interleaved RoPE, causal SDPA, output projection
with column-l2-normalized wo.

Sharding: TP=4 over heads (8 heads/core) x DP=2 over batch across 8 cores.
Each core computes a partial [2048, 2048] output for its batch; host sums
the 4 TP partials per batch.

Key optimizations over the first working version:
- host-side per-partition-contiguous DRAM layouts (64KB descriptors);
  x streamed per 512-row block so the first projection wave starts after
  ~2MB of DMA instead of 16MB.
- projection waves software-pipelined: psum released early via a Pool-engine
  copy; the qT/kT transposes of wave w are emitted after wave w+1's matmuls
  so the PE never waits on the vector norm/rope chain (keeps the PE p-state
  ramped at 2.4 GHz).
- rsqrt via reciprocal_approx_fast (+scalar sqrt) instead of the slow DVE
  reciprocal; rope rotate-half via strided half-views (no rot copies).
- attention: pv psum copied to SBUF by the Pool engine immediately
  (psum ring of 2), denominators batched per block ([8,512] fast recip),
  1/denom broadcast via one packed bf16 matmul per head pair, normalize +
  output projection software-pipelined across blocks (block i+1 attention |
  block i normalize | block i-1 yproj) so the PE stream never breaks.
- mask multiplies, psum->sbuf copies on the otherwise-idle Pool engine.
"""
import sys
import os
import math
from contextlib import ExitStack

sys.path.insert(0, "/opt/trn_rl_repo")

import numpy as np
import ml_dtypes

BF16 = ml_dtypes.bfloat16

B, S, DIM = 2, 2048, 2048
HEADS, DH = 32, 64
THETA = 10000.0
NCORES = 8
TP = 4             # head-parallel ways
HPC = HEADS // TP  # heads per core = 8
E = HPC * DH       # per-core qkv width = 512
ET = E // 128      # e-tiles per core = 4
DT = DIM // 128    # contraction d-tiles = 16
SB = S // 512      # 512-wide seq blocks = 4
SS = S // 128      # 128-wide seq blocks = 16

_CACHE = {}


def _l2n(w, axis):
    n = np.sqrt((w.astype(np.float64) ** 2).sum(axis=axis, keepdims=True))
    n = np.maximum(n, 1e-12)
    return (w / n).astype(np.float32)


def _build_program():
    import concourse.bass as bass
    from concourse import bacc
    import concourse.mybir as mybir
    import concourse.tile as tile
    from concourse.masks import make_identity

    f32 = mybir.dt.float32
    bf16 = mybir.dt.bfloat16
    AF = mybir.ActivationFunctionType
    AX = mybir.AxisListType
    OP = mybir.AluOpType

    nc = bacc.Bacc("TRN2", target_bir_lowering=False)

    xd = nc.dram_tensor("xd", [128, SB * DT * 512], bf16, kind="ExternalInput")
    wqd = nc.dram_tensor("wqd", [128, DT * E], bf16, kind="ExternalInput")
    wkd = nc.dram_tensor("wkd", [128, DT * E], bf16, kind="ExternalInput")
    wvd = nc.dram_tensor("wvd", [128, DT * E], bf16, kind="ExternalInput")
    wod = nc.dram_tensor("wod", [128, ET * DIM], bf16, kind="ExternalInput")
    cosd = nc.dram_tensor("cosd", [128, SS * DH], bf16, kind="ExternalInput")
    sind = nc.dram_tensor("sind", [128, SS * DH], bf16, kind="ExternalInput")
    maskd = nc.dram_tensor("maskd", [128, 4 * 512], bf16, kind="ExternalInput")
    seld = nc.dram_tensor("seld", [33, 128], bf16, kind="ExternalInput")
    Y = nc.dram_tensor("Y", [S, DIM], f32, kind="ExternalOutput")

    with tile.TileContext(nc) as tc, ExitStack() as ctx:
        const = ctx.enter_context(tc.tile_pool(name="const", bufs=1))
        wpool = ctx.enter_context(tc.tile_pool(name="wpool", bufs=1))
        qkv = ctx.enter_context(tc.tile_pool(name="qkv", bufs=1))

        # constants (loaded first; small)
        cos_sb = const.tile([128, SS, DH], bf16)
        sin_sb = const.tile([128, SS, DH], bf16)
        mask_sb = const.tile([128, 4, 512], bf16)
        nc.sync.dma_start(cos_sb, cosd.rearrange("p (b d) -> p b d", d=DH))
        nc.sync.dma_start(sin_sb, sind.rearrange("p (b d) -> p b d", d=DH))
        nc.sync.dma_start(mask_sb, maskd.rearrange("p (r n) -> p r n", n=512))
        ident = const.tile([128, 128], bf16)
        make_identity(nc, ident)
        # head-pair selector for the 1/denom broadcast matmul:
        # sel2[0, 0:64]=1, sel2[1, 64:128]=1
        sel2 = const.tile([33, 128], bf16)
        nc.sync.dma_start(sel2, seld[:, :])


        # persistent activations
        qT = [qkv.tile([128, S], bf16, tag=f"qT{e}", name=f"qT{e}")
              for e in range(ET)]
        kT = [qkv.tile([128, S], bf16, tag=f"kT{e}", name=f"kT{e}")
              for e in range(ET)]
        v_sb = qkv.tile([128, SS, HPC, 65], bf16, tag="v")
        nc.vector.memset(v_sb[:, :, :, 64:65], 1.0)

        # ---------------- projection phase ----------------
        pools = {}

        def emit_mms(w_sb_, x_t, wname):
            pss = [pools["mm"].tile([128, E], f32, tag="mm",
                                    name=f"ps_{wname}_{su}")
                   for su in range(4)]
            for dt in range(DT):
                for su in range(4):
                    nc.tensor.matmul(
                        pss[su], x_t[:, dt, su * 128:(su + 1) * 128],
                        w_sb_[:, dt, :],
                        start=(dt == 0), stop=(dt == DT - 1))
            return pss

        def emit_chain_qk(ps_s, st, su):
            """norm + rope for one si-128 block; returns bf16 qo tile."""
            sblk = st * 4 + su
            sq = pools["work"].tile([128, E], f32, tag="sq", bufs=2)
            nc.scalar.square(sq, ps_s)
            ssq = pools["work"].tile([128, HPC], f32, tag="ssq", bufs=2)
            nc.vector.tensor_reduce(
                ssq, sq.rearrange("p (h d) -> p h d", d=DH),
                axis=AX.X, op=OP.add)
            rr = pools["work"].tile([128, HPC], f32, tag="rr", bufs=2)
            nc.vector.reciprocal_approx_fast(rr, ssq)
            inv = pools["work"].tile([128, HPC], f32, tag="inv", bufs=2)
            nc.scalar.sqrt(inv, rr)  # 1/|q| per (row, head)
            qn = pools["work"].tile([128, HPC, DH], f32, tag="qn", bufs=2)
            nc.gpsimd.tensor_mul(
                qn, ps_s.rearrange("p (h d) -> p h d", d=DH),
                inv.unsqueeze(2).broadcast_to([128, HPC, DH]))
            # rope: q' = qn*cos + swap32(qn)*sin  (sign folded into sin table)
            cosb = cos_sb[:, sblk:sblk + 1, :].broadcast_to([128, HPC, DH])
            qf = pools["work"].tile([128, HPC, DH], f32, tag="qf", bufs=2)
            nc.vector.tensor_mul(qf, qn, cosb)
            qn4 = qn.rearrange("p h (t u) -> p h t u", u=32)
            sin4 = sin_sb[:, sblk:sblk + 1, :].rearrange(
                "p b (t u) -> p b t u", u=32)
            tmp = pools["work"].tile([128, HPC, 2, 32], f32, tag="tmp", bufs=2)
            nc.vector.tensor_mul(
                tmp[:, :, 0:1, :], qn4[:, :, 1:2, :],
                sin4[:, :, 0:1, :].broadcast_to([128, HPC, 1, 32]))
            nc.vector.tensor_mul(
                tmp[:, :, 1:2, :], qn4[:, :, 0:1, :],
                sin4[:, :, 1:2, :].broadcast_to([128, HPC, 1, 32]))
            qo = pools["work"].tile([128, E], bf16, tag="qo", bufs=3)
            nc.vector.tensor_add(
                qo, qf.rearrange("p h d -> p (h d)"),
                tmp.rearrange("p h t u -> p (h t u)"))
            return qo

        def emit_transposes(dstT, st, qos):
            for su, qo in enumerate(qos):
                sblk = st * 4 + su
                for et in range(ET):
                    trp = pools["tr"].tile([128, 128], bf16, tag="tr",
                                           name=f"tr{sblk}_{et}")
                    nc.tensor.transpose(trp, qo[:, et * 128:(et + 1) * 128],
                                        ident)
                    nc.vector.tensor_copy(
                        dstT[et][:, sblk * 128:(sblk + 1) * 128], trp)

        with tc.tile_pool(name="qkvw", bufs=1) as qkvw, \
             tc.tile_pool(name="xpool", bufs=2) as xpool, \
             tc.tile_pool(name="work", bufs=1) as work, \
             tc.tile_pool(name="pr_mm", bufs=6, space="PSUM") as mm_pool, \
             tc.tile_pool(name="pr_tr", bufs=2, space="PSUM") as tr_pool:
            pools["mm"] = mm_pool
            pools["tr"] = tr_pool
            pools["work"] = work

            # weights: wq first (first wave), wo into the persistent pool
            wq_sb = qkvw.tile([128, DT, E], bf16, tag="wq")
            nc.sync.dma_start(wq_sb.rearrange("p t e -> p (t e)"), wqd[:, :])

            def load_x(st):
                xt = xpool.tile([128, DT, 512], bf16, tag="x", name=f"x{st}")
                nc.sync.dma_start(xt.rearrange("p t s -> p (t s)"),
                                  xd[:, st * DT * 512:(st + 1) * DT * 512])
                return xt

            x_cur = load_x(0)
            wk_sb = qkvw.tile([128, DT, E], bf16, tag="wk")
            wv_sb = qkvw.tile([128, DT, E], bf16, tag="wv")
            wo_sb = wpool.tile([128, ET, DIM], bf16, tag="wo")
            nc.sync.dma_start(wk_sb.rearrange("p t e -> p (t e)"), wkd[:, :])
            nc.sync.dma_start(wv_sb.rearrange("p t e -> p (t e)"), wvd[:, :])
            nc.sync.dma_start(wo_sb.rearrange("p t e -> p (t e)"), wod[:, :])

            pending = None  # (dstT, st, qos) transposes deferred one wave
            for st in range(SB):
                x_nxt = load_x(st + 1) if st + 1 < SB else None
                for wname, w_sb_, dstT in ((f"q{st}", wq_sb, qT),
                                           (f"k{st}", wk_sb, kT),
                                           (f"v{st}", wv_sb, None)):
                    pss = emit_mms(w_sb_, x_cur, wname)
                    if dstT is None:
                        for su in range(4):
                            nc.scalar.copy(
                                v_sb[:, st * 4 + su, :, 0:64],
                                pss[su].rearrange("p (h d) -> p h d", d=DH))
                        qos = None
                    else:
                        # release psum banks first (Pool is in-order; the
                        # chains below stall it on ACT results)
                        ps_ss = []
                        for su in range(4):
                            ps_s = pools["work"].tile(
                                [128, E], f32, tag="pss",
                                bufs=4, name=f"pss_{wname}_{su}")
                            nc.scalar.copy(ps_s, pss[su])
                            ps_ss.append(ps_s)
                        qos = [emit_chain_qk(ps_ss[su], st, su)
                               for su in range(4)]
                    if pending is not None:
                        emit_transposes(*pending)
                        pending = None
                    if qos is not None:
                        pending = (dstT, st, qos)
                x_cur = x_nxt
            if pending is not None:
                emit_transposes(*pending)

        # ---------------- attention + output projection ----------------
        def attn_head(h, i, pv):
            """head h, si-512 block i; accumulates into held pv psum.
            row 64 of pv collects softmax denominators (ones column in v)."""
            et, hp = h // 2, (h % 2) * 64
            npr = 2 * (i + 1)
            lgs = {}

            def emit_lg(p):
                lg2 = pools["lg"].tile([128, 2, 512], f32, tag="lg",
                                       name=f"lg{i}_{h}_{p}")
                for bb in range(2):
                    sjb = 2 * p + bb
                    nc.tensor.matmul(
                        lg2[:, bb, :],
                        kT[et][hp:hp + 64, sjb * 128:(sjb + 1) * 128],
                        qT[et][hp:hp + 64, i * 512:(i + 1) * 512],
                        start=True, stop=True)
                lgs[p] = lg2

            emit_lg(0)
            if npr > 1:
                emit_lg(1)
            for p in range(npr):
                lg2 = lgs.pop(p)
                ex = pools["attnp"].tile([128, 2, 512], bf16, tag="ex", bufs=3)
                nc.scalar.activation(ex, lg2, AF.Exp)
                if p + 2 < npr:
                    emit_lg(p + 2)
                for bb in range(2):
                    sjb = 2 * p + bb
                    r = sjb - 4 * i
                    if r >= 0:
                        nc.gpsimd.tensor_mul(ex[:, bb, :], ex[:, bb, :],
                                             mask_sb[:, r, :])
                    nc.tensor.matmul(
                        pv[0:65, :],
                        v_sb[:, sjb, h, :],
                        ex[:, bb, :],
                        start=(sjb == 0), stop=(sjb == 4 * i + 3))

        def normalize_pair(pvs_list, inv8b, i, j):
            """divide heads 2j,2j+1 of block i by softmax denominators."""
            bc = pools["yb"].tile([128, 512], f32, tag="yb",
                                  name=f"bc{i}_{j}")
            nc.tensor.matmul(bc, sel2, inv8b[:, j, :],
                             start=True, stop=True)
            for kk in range(2):
                h = 2 * j + kk
                et, hp = h // 2, (h % 2) * 64
                nc.vector.tensor_mul(
                    qT[et][hp:hp + 64, i * 512:(i + 1) * 512],
                    pvs_list[h][0:64, :], bc[kk * 64:(kk + 1) * 64, :])

        def yproj(ib):
            """si-128 block ib -> Y[ib*128:+128, :]."""
            for nd in range(4):
                ps = pools["yb"].tile([128, 512], f32, tag="yb",
                                      name=f"yps{ib}_{nd}")
                for ket in range(ET):
                    nc.tensor.matmul(
                        ps,
                        qT[ket][:, ib * 128:(ib + 1) * 128],
                        wo_sb[:, ket, nd * 512:(nd + 1) * 512],
                        start=(ket == 0), stop=(ket == ET - 1))
                ys = pools["ypool"].tile([128, 512], f32, tag="y", bufs=3)
                nc.vector.tensor_copy(ys, ps)
                nc.sync.dma_start(
                    Y[ib * 128:(ib + 1) * 128, nd * 512:(nd + 1) * 512], ys)

        with tc.tile_pool(name="attnp", bufs=1) as attnp, \
             tc.tile_pool(name="ypool", bufs=1) as ypool, \
             tc.tile_pool(name="ps_lg", bufs=2, space="PSUM") as lg_pool, \
             tc.tile_pool(name="ps_pv", bufs=2, space="PSUM") as pv_pool, \
             tc.tile_pool(name="ps_yb", bufs=2, space="PSUM") as yb_pool:
            pools["lg"] = lg_pool
            pools["pv"] = pv_pool
            pools["yb"] = yb_pool
            pools["attnp"] = attnp
            pools["ypool"] = ypool
            prev = None  # (i, pvs_list, inv8b) of previous block
            dn8 = pools["attnp"].tile([33, 4, 512], f32, tag="dn8", bufs=1,
                                      name="dn8")
            nc.vector.memset(dn8, 1.0)  # rows 1..31 stay 1.0 (recip-safe)
            for i in range(SB):
                cur_pvs = []
                for h in range(HPC):
                    pv = pv_pool.tile([128, 512], f32, tag="pv",
                                      name=f"pv{i}_{h}")
                    attn_head(h, i, pv)
                    pvs = pools["attnp"].tile([65, 512], f32, tag="pvs", bufs=12,
                                     name=f"pvs{i}_{h}")
                    nc.vector.tensor_copy(pvs, pv[0:65, :])  # frees psum
                    pr = (h % 2) * 32
                    nc.gpsimd.tensor_copy(
                        dn8[pr:pr + 1, h // 2, :], pvs[64:65, :])
                    cur_pvs.append(pvs)
                    if h % 2 == 1 and prev is not None:
                        j = h // 2
                        pi, ppvs, pinv = prev
                        normalize_pair(ppvs, pinv, pi, j)
                        if pi > 0:
                            yproj(4 * (pi - 1) + j)
                inv8 = pools["attnp"].tile([33, 4, 512], f32, tag="inv8", bufs=1,
                                  name=f"inv8_{i}")
                nc.vector.reciprocal_approx_fast(inv8, dn8)
                inv8b = pools["attnp"].tile([33, 4, 512], bf16, tag="inv8b", bufs=2,
                                   name=f"inv8b_{i}")
                nc.gpsimd.tensor_copy(inv8b, inv8)
                prev = (i, cur_pvs, inv8b)
            # flush: normalize block 3, then yproj blocks 2 and 3
            pi, ppvs, pinv = prev
            for j in range(4):
                normalize_pair(ppvs, pinv, pi, j)
                yproj(4 * (pi - 1) + j)
            for ib in range(4 * pi, 4 * pi + 4):
                yproj(ib)

    return nc


def _host_prep(x, wq, wk, wv, wo, qk_scale):
    """Returns per-core input dicts."""
    perm = np.concatenate([np.arange(0, DH, 2), np.arange(1, DH, 2)])
    wq_n = _l2n(wq, -1).reshape(HEADS, DH, DIM)[:, perm, :].reshape(HEADS * DH, DIM)
    wk_n = _l2n(wk, -1).reshape(HEADS, DH, DIM)[:, perm, :].reshape(HEADS * DH, DIM)
    wv_n = _l2n(wv, -1)
    wo_n = _l2n(wo, 0)
    sp = qk_scale.astype(np.float64)[perm]

    # rope tables with qk_scale folded in; permuted-block layout
    half = np.arange(0, DH, 2)
    freqs = 1.0 / (THETA ** (half.astype(np.float64) / DH))      # (32,)
    ang = np.arange(S, dtype=np.float64)[:, None] * freqs[None]  # (S, 32)
    cos_h, sin_h = np.cos(ang), np.sin(ang)
    cos_p = np.concatenate([cos_h, cos_h], 1)                    # (S, 64)
    sin_e = np.concatenate([-sin_h, sin_h], 1)
    cos_eff = (cos_p * sp[None, :]).astype(np.float32)
    swap_sp = np.concatenate([sp[32:], sp[:32]])
    sin_eff = (sin_e * swap_sp[None, :]).astype(np.float32)
    # device layout [128, SS*DH]: [p, b*64+c] = tbl[b*128+p, c]
    cosd = np.ascontiguousarray(
        cos_eff.reshape(SS, 128, DH).transpose(1, 0, 2).reshape(128, SS * DH))
    sind = np.ascontiguousarray(
        sin_eff.reshape(SS, 128, DH).transpose(1, 0, 2).reshape(128, SS * DH))

    # causal masks for the 4 diagonal offsets: keep sjl + 128r <= sil
    sjl = np.arange(128)[:, None]
    sil = np.arange(512)[None, :]
    maskd = np.ascontiguousarray(np.concatenate(
        [(sjl + 128 * r <= sil).astype(np.float32) for r in range(4)],
        axis=1))  # [128, 4*512]

    def wlay(a):  # [E, DIM] -> [128, DT*E] with [p, t*E+e] = a[e, t*128+p]
        return np.ascontiguousarray(
            a.T.reshape(DT, 128, E).transpose(1, 0, 2).reshape(128, DT * E))

    sel2h = np.zeros((33, 128), dtype=np.float32)
    sel2h[0, 0:64] = 1.0
    sel2h[32, 64:128] = 1.0

    in_maps = []
    for c in range(NCORES):
        b, t = divmod(c, TP)
        e0 = t * E
        xb = x[b]  # [S, DIM]
        xdev = np.ascontiguousarray(
            xb.reshape(SB, 512, DT, 128).transpose(3, 0, 2, 1)
            .reshape(128, SB * DT * 512))
        wob = wo_n[:, e0:e0 + E]  # [DIM, E]
        wodev = np.ascontiguousarray(
            wob.T.reshape(ET, 128, DIM).transpose(1, 0, 2)
            .reshape(128, ET * DIM))
        in_maps.append({
            "xd": xdev.astype(BF16),
            "wqd": wlay(wq_n[e0:e0 + E]).astype(BF16),
            "wkd": wlay(wk_n[e0:e0 + E]).astype(BF16),
            "wvd": wlay(wv_n[e0:e0 + E]).astype(BF16),
            "wod": wodev.astype(BF16),
            "cosd": cosd.astype(BF16), "sind": sind.astype(BF16),
            "seld": sel2h.astype(BF16),
            "maskd": maskd.astype(BF16),
        })
    return in_maps


def _install_profile_hook():
    """antenv.axon_hooks is absent in this image; shim it and register the
    ctypes NTFF hook against /opt/axon/libaxon_pjrt.so (mirrors trn_boot)."""
    import types
    import ctypes
    import contextlib

    try:
        from antenv.axon_hooks import get_axon_ntff_profile_hook  # noqa
        return
    except ImportError:
        pass
    import antenv
    mod = types.ModuleType("antenv.axon_hooks")
    state = {}
    mod.set_axon_ntff_profile_hook = lambda h: state.__setitem__("h", h)
    mod.get_axon_ntff_profile_hook = lambda: state.get("h")
    sys.modules["antenv.axon_hooks"] = mod
    antenv.axon_hooks = mod

    so_path = "/opt/axon/libaxon_pjrt.so"
    lib = ctypes.CDLL(so_path)
    if not hasattr(lib, "axon_start_nrt_profile"):
        return
    lib.axon_start_nrt_profile.argtypes = [
        ctypes.POINTER(ctypes.c_int64), ctypes.c_size_t]
    lib.axon_start_nrt_profile.restype = ctypes.c_int64
    lib.axon_stop_nrt_profile.argtypes = [ctypes.c_char_p]
    lib.axon_stop_nrt_profile.restype = ctypes.c_int64

    @contextlib.contextmanager
    def _hook(output_dir, device_ids):
        import jax
        jax.devices()
        if device_ids:
            ids = (ctypes.c_int64 * len(device_ids))(*device_ids)
            rc = lib.axon_start_nrt_profile(ids, len(device_ids))
        else:
            rc = lib.axon_start_nrt_profile(None, 0)
        if rc != 0:
            raise RuntimeError(f"axon_start_nrt_profile rc={rc}")
        try:
            yield
        finally:
            n = lib.axon_stop_nrt_profile(str(output_dir).encode())
            print(f"profile: {n} file(s) written to {output_dir}",
                  file=sys.stderr)

    mod.set_axon_ntff_profile_hook(_hook)


def kernel(x, wq, wk, wv, wo, qk_scale, _profile=False):
    from concourse.bass_utils import run_bass_kernel_spmd

    if _profile:
        _install_profile_hook()

    if "nc" not in _CACHE:
        nc = _build_program()
        nc.finalize()
        _CACHE["nc"] = nc
    nc = _CACHE["nc"]
    in_maps = _host_prep(np.asarray(x), np.asarray(wq), np.asarray(wk),
                         np.asarray(wv), np.asarray(wo), np.asarray(qk_scale))
    res = run_bass_kernel_spmd(nc, in_maps, core_ids=list(range(NCORES)),
                               trace=_profile)
    outs = res.results
    y = np.empty((B, S, DIM), dtype=np.float32)
    for b in range(B):
        y[b] = sum(outs[b * TP + t]["Y"] for t in range(TP))
    if _profile:
        _CACHE["last_exec_time_ns"] = res.exec_time_ns
        _CACHE["last_profile"] = res.profile_json
    return y


# revision 18
# speedup vs baseline: 1.1264x; 1.1264x over previous
"""nn_Attention Trainium2 Bass kernel (optimized).

Full attention forward: x->(q,k,v) with l2-normalized weights, per-head-dim
l2 norm + learned qk scale, interleaved RoPE, causal SDPA, output projection
with column-l2-normalized wo.

Sharding: TP=4 over heads (8 heads/core) x DP=2 over batch across 8 cores.
Each core computes a partial [2048, 2048] output for its batch; host sums
the 4 TP partials per batch.

Key optimizations over the first working version:
- host-side per-partition-contiguous DRAM layouts (64KB descriptors);
  x streamed per 512-row block so the first projection wave starts after
  ~2MB of DMA instead of 16MB.
- projection waves software-pipelined: psum released early via a Pool-engine
  copy; the qT/kT transposes of wave w are emitted after wave w+1's matmuls
  so the PE never waits on the vector norm/rope chain (keeps the PE p-state
  ramped at 2.4 GHz).
- rsqrt via reciprocal_approx_fast (+scalar sqrt) instead of the slow DVE
  reciprocal; rope rotate-half via strided half-views (no rot copies).
- attention: pv psum copied to SBUF by the Pool engine immediately
  (psum ring of 2), denominators batched per block ([8,512] fast recip),
  1/denom broadcast via one packed bf16 matmul per head pair, normalize +
  output projection software-pipelined across blocks (block i+1 attention |
  block i normalize | block i-1 yproj) so the PE stream never breaks.
- mask multiplies, psum->sbuf copies on the otherwise-idle Pool engine.
"""
import sys
import os
import math
from contextlib import ExitStack

sys.path.insert(0, "/opt/trn_rl_repo")

import numpy as np
import ml_dtypes

BF16 = ml_dtypes.bfloat16

B, S, DIM = 2, 2048, 2048
HEADS, DH = 32, 64
THETA = 10000.0
NCORES = 8
TP = 4             # head-parallel ways
HPC = HEADS // TP  # heads per core = 8
E = HPC * DH       # per-core qkv width = 512
ET = E // 128      # e-tiles per core = 4
DT = DIM // 128    # contraction d-tiles = 16
SB = S // 512      # 512-wide seq blocks = 4
SS = S // 128      # 128-wide seq blocks = 16

_CACHE = {}


def _l2n(w, axis):
    n = np.sqrt((w.astype(np.float64) ** 2).sum(axis=axis, keepdims=True))
    n = np.maximum(n, 1e-12)
    return (w / n).astype(np.float32)


def _build_program():
    import concourse.bass as bass
    from concourse import bacc
    import concourse.mybir as mybir
    import concourse.tile as tile
    from concourse.masks import make_identity

    f32 = mybir.dt.float32
    bf16 = mybir.dt.bfloat16
    AF = mybir.ActivationFunctionType
    AX = mybir.AxisListType
    OP = mybir.AluOpType

    nc = bacc.Bacc("TRN2", target_bir_lowering=False)

    xd = nc.dram_tensor("xd", [128, SB * DT * 512], bf16, kind="ExternalInput")
    wqd = nc.dram_tensor("wqd", [128, DT * E], bf16, kind="ExternalInput")
    wkd = nc.dram_tensor("wkd", [128, DT * E], bf16, kind="ExternalInput")
    wvd = nc.dram_tensor("wvd", [128, DT * E], bf16, kind="ExternalInput")
    wod = nc.dram_tensor("wod", [128, ET * DIM], bf16, kind="ExternalInput")
    cosd = nc.dram_tensor("cosd", [128, SS * DH], bf16, kind="ExternalInput")
    sind = nc.dram_tensor("sind", [128, SS * DH], bf16, kind="ExternalInput")
    maskd = nc.dram_tensor("maskd", [128, 4 * 512], bf16, kind="ExternalInput")
    seld = nc.dram_tensor("seld", [33, 128], bf16, kind="ExternalInput")
    Y = nc.dram_tensor("Y", [S, DIM], f32, kind="ExternalOutput")

    with tile.TileContext(nc) as tc, ExitStack() as ctx:
        const = ctx.enter_context(tc.tile_pool(name="const", bufs=1))
        wpool = ctx.enter_context(tc.tile_pool(name="wpool", bufs=1))
        qkv = ctx.enter_context(tc.tile_pool(name="qkv", bufs=1))

        # constants (loaded first; small)
        cos_sb = const.tile([128, SS, DH], bf16)
        sin_sb = const.tile([128, SS, DH], bf16)
        mask_sb = const.tile([128, 4, 512], bf16)
        nc.sync.dma_start(cos_sb, cosd.rearrange("p (b d) -> p b d", d=DH))
        nc.sync.dma_start(sin_sb, sind.rearrange("p (b d) -> p b d", d=DH))
        nc.sync.dma_start(mask_sb, maskd.rearrange("p (r n) -> p r n", n=512))
        ident = const.tile([128, 128], bf16)
        make_identity(nc, ident)
        # head-pair selector for the 1/denom broadcast matmul:
        # sel2[0, 0:64]=1, sel2[1, 64:128]=1
        sel2 = const.tile([33, 128], bf16)
        nc.sync.dma_start(sel2, seld[:, :])


        # persistent activations
        qT = [qkv.tile([128, S], bf16, tag=f"qT{e}", name=f"qT{e}")
              for e in range(ET)]
        kT = [qkv.tile([128, S], bf16, tag=f"kT{e}", name=f"kT{e}")
              for e in range(ET)]
        v_sb = qkv.tile([128, SS, HPC, 65], bf16, tag="v")
        nc.vector.memset(v_sb[:, :, :, 64:65], 1.0)

        # ---------------- projection phase ----------------
        pools = {}

        def emit_mms(w_sb_, x_t, wname):
            pss = [pools["mm"].tile([128, E], f32, tag="mm",
                                    name=f"ps_{wname}_{su}")
                   for su in range(4)]
            for dt in range(DT):
                for su in range(4):
                    nc.tensor.matmul(
                        pss[su], x_t[:, dt, su * 128:(su + 1) * 128],
                        w_sb_[:, dt, :],
                        start=(dt == 0), stop=(dt == DT - 1))
            return pss

        def emit_chain_qk(ps_s, st, su):
            """norm + rope for one si-128 block; returns bf16 qo tile."""
            sblk = st * 4 + su
            sq = pools["work"].tile([128, E], f32, tag="sq", bufs=2)
            nc.scalar.square(sq, ps_s)
            ssq = pools["work"].tile([128, HPC], f32, tag="ssq", bufs=2)
            nc.vector.tensor_reduce(
                ssq, sq.rearrange("p (h d) -> p h d", d=DH),
                axis=AX.X, op=OP.add)
            rr = pools["work"].tile([128, HPC], f32, tag="rr", bufs=2)
            nc.vector.reciprocal_approx_fast(rr, ssq)
            inv = pools["work"].tile([128, HPC], f32, tag="inv", bufs=2)
            nc.scalar.sqrt(inv, rr)  # 1/|q| per (row, head)
            qn = pools["work"].tile([128, HPC, DH], f32, tag="qn", bufs=2)
            nc.gpsimd.tensor_mul(
                qn, ps_s.rearrange("p (h d) -> p h d", d=DH),
                inv.unsqueeze(2).broadcast_to([128, HPC, DH]))
            # rope: q' = qn*cos + swap32(qn)*sin  (sign folded into sin table)
            cosb = cos_sb[:, sblk:sblk + 1, :].broadcast_to([128, HPC, DH])
            qf = pools["work"].tile([128, HPC, DH], f32, tag="qf", bufs=2)
            nc.vector.tensor_mul(qf, qn, cosb)
            qn4 = qn.rearrange("p h (t u) -> p h t u", u=32)
            sin4 = sin_sb[:, sblk:sblk + 1, :].rearrange(
                "p b (t u) -> p b t u", u=32)
            tmp = pools["work"].tile([128, HPC, 2, 32], f32, tag="tmp", bufs=2)
            nc.vector.tensor_mul(
                tmp[:, :, 0:1, :], qn4[:, :, 1:2, :],
                sin4[:, :, 0:1, :].broadcast_to([128, HPC, 1, 32]))
            nc.vector.tensor_mul(
                tmp[:, :, 1:2, :], qn4[:, :, 0:1, :],
                sin4[:, :, 1:2, :].broadcast_to([128, HPC, 1, 32]))
            qo = pools["work"].tile([128, E], bf16, tag="qo", bufs=3)
            nc.vector.tensor_add(
                qo, qf.rearrange("p h d -> p (h d)"),
                tmp.rearrange("p h t u -> p (h t u)"))
            return qo

        def emit_transposes(dstT, st, qos):
            for su, qo in enumerate(qos):
                sblk = st * 4 + su
                for et in range(ET):
                    trp = pools["tr"].tile([128, 128], bf16, tag="tr",
                                           name=f"tr{sblk}_{et}")
                    nc.tensor.transpose(trp, qo[:, et * 128:(et + 1) * 128],
                                        ident)
                    nc.vector.tensor_copy(
                        dstT[et][:, sblk * 128:(sblk + 1) * 128], trp)

        with tc.tile_pool(name="qkvw", bufs=1) as qkvw, \
             tc.tile_pool(name="xpool", bufs=2) as xpool, \
             tc.tile_pool(name="work", bufs=1) as work, \
             tc.tile_pool(name="pr_mm", bufs=6, space="PSUM") as mm_pool, \
             tc.tile_pool(name="pr_tr", bufs=2, space="PSUM") as tr_pool:
            pools["mm"] = mm_pool
            pools["tr"] = tr_pool
            pools["work"] = work

            # weights: wq first (first wave), wo into the persistent pool
            def load_w(dst, srcd, nchunk=4):
                fl = dst.rearrange("p t e -> p (t e)")
                tot = fl.shape[1]
                ch = tot // nchunk
                for c in range(nchunk):
                    nc.sync.dma_start(fl[:, c * ch:(c + 1) * ch],
                                      srcd[:, c * ch:(c + 1) * ch])

            wq_sb = qkvw.tile([128, DT, E], bf16, tag="wq")
            load_w(wq_sb, wqd)

            def load_x(st):
                xt = xpool.tile([128, DT, 512], bf16, tag="x", name=f"x{st}")
                fl = xt.rearrange("p t s -> p (t s)")
                base = st * DT * 512
                ch = DT * 512 // 8
                for c in range(8):
                    nc.sync.dma_start(fl[:, c * ch:(c + 1) * ch],
                                      xd[:, base + c * ch:base + (c + 1) * ch])
                return xt

            x_cur = load_x(0)
            wk_sb = qkvw.tile([128, DT, E], bf16, tag="wk")
            wv_sb = qkvw.tile([128, DT, E], bf16, tag="wv")
            wo_sb = wpool.tile([128, ET, DIM], bf16, tag="wo")
            load_w(wk_sb, wkd)
            load_w(wv_sb, wvd)
            load_w(wo_sb, wod)

            pending = None  # (dstT, st, qos) transposes deferred one wave
            for st in range(SB):
                x_nxt = load_x(st + 1) if st + 1 < SB else None
                for wname, w_sb_, dstT in ((f"q{st}", wq_sb, qT),
                                           (f"k{st}", wk_sb, kT),
                                           (f"v{st}", wv_sb, None)):
                    pss = emit_mms(w_sb_, x_cur, wname)
                    if dstT is None:
                        for su in range(4):
                            eng = nc.scalar.copy if su % 2 == 0 \
                                else nc.vector.tensor_copy
                            eng(v_sb[:, st * 4 + su, :, 0:64],
                                pss[su].rearrange("p (h d) -> p h d", d=DH))
                        qos = None
                    else:
                        # release psum banks first (Pool is in-order; the
                        # chains below stall it on ACT results)
                        ps_ss = []
                        for su in range(4):
                            ps_s = pools["work"].tile(
                                [128, E], f32, tag="pss",
                                bufs=4, name=f"pss_{wname}_{su}")
                            if su % 2 == 0:
                                nc.scalar.copy(ps_s, pss[su])
                            else:
                                nc.vector.tensor_copy(ps_s, pss[su])
                            ps_ss.append(ps_s)
                        qos = [emit_chain_qk(ps_ss[su], st, su)
                               for su in range(4)]
                    if pending is not None:
                        emit_transposes(*pending)
                        pending = None
                    if qos is not None:
                        pending = (dstT, st, qos)
                x_cur = x_nxt
            if pending is not None:
                emit_transposes(*pending)

        # ---------------- attention + output projection ----------------
        def attn_head(h, i, pv):
            """head h, si-512 block i; accumulates into held pv psum.
            row 64 of pv collects softmax denominators (ones column in v)."""
            et, hp = h // 2, (h % 2) * 64
            npr = 2 * (i + 1)
            lgs = {}

            def emit_lg(p):
                lg2 = pools["lg"].tile([128, 2, 512], f32, tag="lg",
                                       name=f"lg{i}_{h}_{p}")
                for bb in range(2):
                    sjb = 2 * p + bb
                    nc.tensor.matmul(
                        lg2[:, bb, :],
                        kT[et][hp:hp + 64, sjb * 128:(sjb + 1) * 128],
                        qT[et][hp:hp + 64, i * 512:(i + 1) * 512],
                        start=True, stop=True)
                lgs[p] = lg2

            emit_lg(0)
            if npr > 1:
                emit_lg(1)
            for p in range(npr):
                lg2 = lgs.pop(p)
                ex = pools["attnp"].tile([128, 2, 512], bf16, tag="ex", bufs=3)
                nc.scalar.activation(ex, lg2, AF.Exp)
                if p + 2 < npr:
                    emit_lg(p + 2)
                for bb in range(2):
                    sjb = 2 * p + bb
                    r = sjb - 4 * i
                    if r >= 0:
                        nc.gpsimd.tensor_mul(ex[:, bb, :], ex[:, bb, :],
                                             mask_sb[:, r, :])
                    nc.tensor.matmul(
                        pv[0:65, :],
                        v_sb[:, sjb, h, :],
                        ex[:, bb, :],
                        start=(sjb == 0), stop=(sjb == 4 * i + 3))

        def normalize_pair(pvs_list, inv8b, i, j):
            """divide heads 2j,2j+1 of block i by softmax denominators."""
            bc = pools["yb"].tile([128, 512], f32, tag="yb",
                                  name=f"bc{i}_{j}")
            nc.tensor.matmul(bc, sel2, inv8b[:, j, :],
                             start=True, stop=True)
            for kk in range(2):
                h = 2 * j + kk
                et, hp = h // 2, (h % 2) * 64
                nc.vector.tensor_mul(
                    qT[et][hp:hp + 64, i * 512:(i + 1) * 512],
                    pvs_list[h][0:64, :], bc[kk * 64:(kk + 1) * 64, :])

        def yproj(ib):
            """si-128 block ib -> Y[ib*128:+128, :]."""
            for nd in range(4):
                ps = pools["yb"].tile([128, 512], f32, tag="yb",
                                      name=f"yps{ib}_{nd}")
                for ket in range(ET):
                    nc.tensor.matmul(
                        ps,
                        qT[ket][:, ib * 128:(ib + 1) * 128],
                        wo_sb[:, ket, nd * 512:(nd + 1) * 512],
                        start=(ket == 0), stop=(ket == ET - 1))
                ys = pools["ypool"].tile([128, 512], f32, tag="y", bufs=4)
                if nd % 2 == 0:
                    nc.scalar.copy(ys, ps)
                else:
                    nc.vector.tensor_copy(ys, ps)
                nc.sync.dma_start(
                    Y[ib * 128:(ib + 1) * 128, nd * 512:(nd + 1) * 512], ys)

        with tc.tile_pool(name="attnp", bufs=1) as attnp, \
             tc.tile_pool(name="ypool", bufs=1) as ypool, \
             tc.tile_pool(name="ps_lg", bufs=2, space="PSUM") as lg_pool, \
             tc.tile_pool(name="ps_pv", bufs=2, space="PSUM") as pv_pool, \
             tc.tile_pool(name="ps_yb", bufs=2, space="PSUM") as yb_pool:
            pools["lg"] = lg_pool
            pools["pv"] = pv_pool
            pools["yb"] = yb_pool
            pools["attnp"] = attnp
            pools["ypool"] = ypool
            prev = None  # (i, pvs_list, inv8b) of previous block
            dn8 = pools["attnp"].tile([33, 4, 512], f32, tag="dn8", bufs=1,
                                      name="dn8")
            nc.vector.memset(dn8, 1.0)  # rows 1..31 stay 1.0 (recip-safe)
            for i in range(SB):
                cur_pvs = []
                for h in range(HPC):
                    pv = pv_pool.tile([128, 512], f32, tag="pv",
                                      name=f"pv{i}_{h}")
                    attn_head(h, i, pv)
                    pvs = pools["attnp"].tile([65, 512], f32, tag="pvs", bufs=12,
                                     name=f"pvs{i}_{h}")
                    nc.vector.tensor_copy(pvs, pv[0:65, :])  # frees psum
                    pr = (h % 2) * 32
                    nc.vector.tensor_copy(
                        dn8[pr:pr + 1, h // 2, :], pvs[64:65, :])
                    cur_pvs.append(pvs)
                    if h % 2 == 1 and prev is not None:
                        j = h // 2
                        pi, ppvs, pinv = prev
                        normalize_pair(ppvs, pinv, pi, j)
                        if pi > 0:
                            yproj(4 * (pi - 1) + j)
                inv8 = pools["attnp"].tile([33, 4, 512], f32, tag="inv8", bufs=1,
                                  name=f"inv8_{i}")
                nc.vector.reciprocal_approx_fast(inv8, dn8)
                inv8b = pools["attnp"].tile([33, 4, 512], bf16, tag="inv8b", bufs=2,
                                   name=f"inv8b_{i}")
                nc.scalar.copy(inv8b, inv8)
                prev = (i, cur_pvs, inv8b)
            # flush: yproj block 2 first (fills PE while the last
            # denominators come through recip/cast), then block 3
            pi, ppvs, pinv = prev
            for j in range(4):
                yproj(4 * (pi - 1) + j)
                normalize_pair(ppvs, pinv, pi, j)
            for ib in range(4 * pi, 4 * pi + 4):
                yproj(ib)

    return nc


def _host_prep(x, wq, wk, wv, wo, qk_scale):
    """Returns per-core input dicts."""
    perm = np.concatenate([np.arange(0, DH, 2), np.arange(1, DH, 2)])
    wq_n = _l2n(wq, -1).reshape(HEADS, DH, DIM)[:, perm, :].reshape(HEADS * DH, DIM)
    wk_n = _l2n(wk, -1).reshape(HEADS, DH, DIM)[:, perm, :].reshape(HEADS * DH, DIM)
    wv_n = _l2n(wv, -1)
    wo_n = _l2n(wo, 0)
    sp = qk_scale.astype(np.float64)[perm]

    # rope tables with qk_scale folded in; permuted-block layout
    half = np.arange(0, DH, 2)
    freqs = 1.0 / (THETA ** (half.astype(np.float64) / DH))      # (32,)
    ang = np.arange(S, dtype=np.float64)[:, None] * freqs[None]  # (S, 32)
    cos_h, sin_h = np.cos(ang), np.sin(ang)
    cos_p = np.concatenate([cos_h, cos_h], 1)                    # (S, 64)
    sin_e = np.concatenate([-sin_h, sin_h], 1)
    cos_eff = (cos_p * sp[None, :]).astype(np.float32)
    swap_sp = np.concatenate([sp[32:], sp[:32]])
    sin_eff = (sin_e * swap_sp[None, :]).astype(np.float32)
    # device layout [128, SS*DH]: [p, b*64+c] = tbl[b*128+p, c]
    cosd = np.ascontiguousarray(
        cos_eff.reshape(SS, 128, DH).transpose(1, 0, 2).reshape(128, SS * DH))
    sind = np.ascontiguousarray(
        sin_eff.reshape(SS, 128, DH).transpose(1, 0, 2).reshape(128, SS * DH))

    # causal masks for the 4 diagonal offsets: keep sjl + 128r <= sil
    sjl = np.arange(128)[:, None]
    sil = np.arange(512)[None, :]
    maskd = np.ascontiguousarray(np.concatenate(
        [(sjl + 128 * r <= sil).astype(np.float32) for r in range(4)],
        axis=1))  # [128, 4*512]

    def wlay(a):  # [E, DIM] -> [128, DT*E] with [p, t*E+e] = a[e, t*128+p]
        return np.ascontiguousarray(
            a.T.reshape(DT, 128, E).transpose(1, 0, 2).reshape(128, DT * E))

    sel2h = np.zeros((33, 128), dtype=np.float32)
    sel2h[0, 0:64] = 1.0
    sel2h[32, 64:128] = 1.0

    in_maps = []
    for c in range(NCORES):
        b, t = divmod(c, TP)
        e0 = t * E
        xb = x[b]  # [S, DIM]
        xdev = np.ascontiguousarray(
            xb.reshape(SB, 512, DT, 128).transpose(3, 0, 2, 1)
            .reshape(128, SB * DT * 512))
        wob = wo_n[:, e0:e0 + E]  # [DIM, E]
        wodev = np.ascontiguousarray(
            wob.T.reshape(ET, 128, DIM).transpose(1, 0, 2)
            .reshape(128, ET * DIM))
        in_maps.append({
            "xd": xdev.astype(BF16),
            "wqd": wlay(wq_n[e0:e0 + E]).astype(BF16),
            "wkd": wlay(wk_n[e0:e0 + E]).astype(BF16),
            "wvd": wlay(wv_n[e0:e0 + E]).astype(BF16),
            "wod": wodev.astype(BF16),
            "cosd": cosd.astype(BF16), "sind": sind.astype(BF16),
            "seld": sel2h.astype(BF16),
            "maskd": maskd.astype(BF16),
        })
    return in_maps


def _install_profile_hook():
    """antenv.axon_hooks is absent in this image; shim it and register the
    ctypes NTFF hook against /opt/axon/libaxon_pjrt.so (mirrors trn_boot)."""
    import types
    import ctypes
    import contextlib

    try:
        from antenv.axon_hooks import get_axon_ntff_profile_hook  # noqa
        return
    except ImportError:
        pass
    import antenv
    mod = types.ModuleType("antenv.axon_hooks")
    state = {}
    mod.set_axon_ntff_profile_hook = lambda h: state.__setitem__("h", h)
    mod.get_axon_ntff_profile_hook = lambda: state.get("h")
    sys.modules["antenv.axon_hooks"] = mod
    antenv.axon_hooks = mod

    so_path = "/opt/axon/libaxon_pjrt.so"
    lib = ctypes.CDLL(so_path)
    if not hasattr(lib, "axon_start_nrt_profile"):
        return
    lib.axon_start_nrt_profile.argtypes = [
        ctypes.POINTER(ctypes.c_int64), ctypes.c_size_t]
    lib.axon_start_nrt_profile.restype = ctypes.c_int64
    lib.axon_stop_nrt_profile.argtypes = [ctypes.c_char_p]
    lib.axon_stop_nrt_profile.restype = ctypes.c_int64

    @contextlib.contextmanager
    def _hook(output_dir, device_ids):
        import jax
        jax.devices()
        if device_ids:
            ids = (ctypes.c_int64 * len(device_ids))(*device_ids)
            rc = lib.axon_start_nrt_profile(ids, len(device_ids))
        else:
            rc = lib.axon_start_nrt_profile(None, 0)
        if rc != 0:
            raise RuntimeError(f"axon_start_nrt_profile rc={rc}")
        try:
            yield
        finally:
            n = lib.axon_stop_nrt_profile(str(output_dir).encode())
            print(f"profile: {n} file(s) written to {output_dir}",
                  file=sys.stderr)

    mod.set_axon_ntff_profile_hook(_hook)


def kernel(x, wq, wk, wv, wo, qk_scale, _profile=False):
    from concourse.bass_utils import run_bass_kernel_spmd

    if _profile:
        _install_profile_hook()

    if "nc" not in _CACHE:
        nc = _build_program()
        nc.finalize()
        _CACHE["nc"] = nc
    nc = _CACHE["nc"]
    in_maps = _host_prep(np.asarray(x), np.asarray(wq), np.asarray(wk),
                         np.asarray(wv), np.asarray(wo), np.asarray(qk_scale))
    res = run_bass_kernel_spmd(nc, in_maps, core_ids=list(range(NCORES)),
                               trace=_profile)
    outs = res.results
    y = np.empty((B, S, DIM), dtype=np.float32)
    for b in range(B):
        y[b] = sum(outs[b * TP + t]["Y"] for t in range(TP))
    if _profile:
        _CACHE["last_exec_time_ns"] = res.exec_time_ns
        _CACHE["last_profile"] = res.profile_json
    return y


# revision 19
# speedup vs baseline: 1.2146x; 1.0783x over previous
"""nn_Attention Trainium2 Bass kernel (optimized).

Full attention forward: x->(q,k,v) with l2-normalized weights, per-head-dim
l2 norm + learned qk scale, interleaved RoPE, causal SDPA, output projection
with column-l2-normalized wo.

Sharding: TP=4 over heads (8 heads/core) x DP=2 over batch across 8 cores.
Each core computes a partial [2048, 2048] output for its batch; host sums
the 4 TP partials per batch.

Key optimizations over the first working version:
- host-side per-partition-contiguous DRAM layouts (64KB descriptors);
  x streamed per 512-row block so the first projection wave starts after
  ~2MB of DMA instead of 16MB.
- projection waves software-pipelined: psum released early via a Pool-engine
  copy; the qT/kT transposes of wave w are emitted after wave w+1's matmuls
  so the PE never waits on the vector norm/rope chain (keeps the PE p-state
  ramped at 2.4 GHz).
- rsqrt via reciprocal_approx_fast (+scalar sqrt) instead of the slow DVE
  reciprocal; rope rotate-half via strided half-views (no rot copies).
- attention: pv psum copied to SBUF by the Pool engine immediately
  (psum ring of 2), denominators batched per block ([8,512] fast recip),
  1/denom broadcast via one packed bf16 matmul per head pair, normalize +
  output projection software-pipelined across blocks (block i+1 attention |
  block i normalize | block i-1 yproj) so the PE stream never breaks.
- mask multiplies, psum->sbuf copies on the otherwise-idle Pool engine.
"""
import sys
import os
import math
from contextlib import ExitStack

sys.path.insert(0, "/opt/trn_rl_repo")

import numpy as np
import ml_dtypes

BF16 = ml_dtypes.bfloat16

B, S, DIM = 2, 2048, 2048
HEADS, DH = 32, 64
THETA = 10000.0
NCORES = 8
TP = 4             # head-parallel ways
HPC = HEADS // TP  # heads per core = 8
E = HPC * DH       # per-core qkv width = 512
ET = E // 128      # e-tiles per core = 4
DT = DIM // 128    # contraction d-tiles = 16
SB = S // 512      # 512-wide seq blocks = 4
SS = S // 128      # 128-wide seq blocks = 16

_CACHE = {}


def _l2n(w, axis):
    n = np.sqrt((w.astype(np.float64) ** 2).sum(axis=axis, keepdims=True))
    n = np.maximum(n, 1e-12)
    return (w / n).astype(np.float32)


def _build_program():
    import concourse.bass as bass
    from concourse import bacc
    import concourse.mybir as mybir
    import concourse.tile as tile
    from concourse.masks import make_identity

    f32 = mybir.dt.float32
    bf16 = mybir.dt.bfloat16
    AF = mybir.ActivationFunctionType
    AX = mybir.AxisListType
    OP = mybir.AluOpType

    nc = bacc.Bacc("TRN2", target_bir_lowering=False)

    xd = nc.dram_tensor("xd", [128, SB * DT * 512], bf16, kind="ExternalInput")
    wqd = nc.dram_tensor("wqd", [128, DT * E], bf16, kind="ExternalInput")
    wkd = nc.dram_tensor("wkd", [128, DT * E], bf16, kind="ExternalInput")
    wvd = nc.dram_tensor("wvd", [128, DT * E], bf16, kind="ExternalInput")
    wod = nc.dram_tensor("wod", [128, ET * DIM], bf16, kind="ExternalInput")
    cosd = nc.dram_tensor("cosd", [128, SS * DH], bf16, kind="ExternalInput")
    sind = nc.dram_tensor("sind", [128, SS * DH], bf16, kind="ExternalInput")
    maskd = nc.dram_tensor("maskd", [128, 4 * 512], bf16, kind="ExternalInput")
    seld = nc.dram_tensor("seld", [33, 128], bf16, kind="ExternalInput")
    Y = nc.dram_tensor("Y", [S, DIM], f32, kind="ExternalOutput")

    with tile.TileContext(nc) as tc, ExitStack() as ctx:
        const = ctx.enter_context(tc.tile_pool(name="const", bufs=1))
        wpool = ctx.enter_context(tc.tile_pool(name="wpool", bufs=1))
        qkv = ctx.enter_context(tc.tile_pool(name="qkv", bufs=1))

        # constants (loaded first; small)
        cos_sb = const.tile([128, SS, DH], bf16)
        sin_sb = const.tile([128, SS, DH], bf16)
        mask_sb = const.tile([128, 4, 512], bf16)
        nc.sync.dma_start(cos_sb, cosd.rearrange("p (b d) -> p b d", d=DH))
        nc.sync.dma_start(sin_sb, sind.rearrange("p (b d) -> p b d", d=DH))
        nc.sync.dma_start(mask_sb, maskd.rearrange("p (r n) -> p r n", n=512))
        ident = const.tile([128, 128], bf16)
        make_identity(nc, ident)
        # head-pair selector for the 1/denom broadcast matmul:
        # sel2[0, 0:64]=1, sel2[1, 64:128]=1
        sel2 = const.tile([33, 128], bf16)
        nc.sync.dma_start(sel2, seld[:, :])


        # persistent activations
        qT = [qkv.tile([128, S], bf16, tag=f"qT{e}", name=f"qT{e}")
              for e in range(ET)]
        kT = [qkv.tile([128, S], bf16, tag=f"kT{e}", name=f"kT{e}")
              for e in range(ET)]
        v_sb = qkv.tile([128, SS, HPC, 65], bf16, tag="v")
        nc.vector.memset(v_sb[:, :, :, 64:65], 1.0)

        # ---------------- projection phase ----------------
        pools = {}

        def emit_mms(w_sb_, x_t, wname):
            pss = [pools["mm"].tile([128, E], f32, tag="mm",
                                    name=f"ps_{wname}_{su}")
                   for su in range(4)]
            for dt in range(DT):
                for su in range(4):
                    nc.tensor.matmul(
                        pss[su], x_t[:, dt, su * 128:(su + 1) * 128],
                        w_sb_[:, dt, :],
                        start=(dt == 0), stop=(dt == DT - 1))
            return pss

        def emit_chain_qk(ps_s, st, su):
            """norm + rope for one si-128 block; returns bf16 qo tile."""
            sblk = st * 4 + su
            sq = pools["work"].tile([128, E], f32, tag="sq", bufs=2)
            nc.scalar.square(sq, ps_s)
            ssq = pools["work"].tile([128, HPC], f32, tag="ssq", bufs=2)
            nc.vector.tensor_reduce(
                ssq, sq.rearrange("p (h d) -> p h d", d=DH),
                axis=AX.X, op=OP.add)
            rr = pools["work"].tile([128, HPC], f32, tag="rr", bufs=2)
            nc.vector.reciprocal_approx_fast(rr, ssq)
            inv = pools["work"].tile([128, HPC], f32, tag="inv", bufs=2)
            nc.scalar.sqrt(inv, rr)  # 1/|q| per (row, head)
            qn = pools["work"].tile([128, HPC, DH], f32, tag="qn", bufs=2)
            nc.gpsimd.tensor_mul(
                qn, ps_s.rearrange("p (h d) -> p h d", d=DH),
                inv.unsqueeze(2).broadcast_to([128, HPC, DH]))
            # rope: q' = qn*cos + swap32(qn)*sin  (sign folded into sin table)
            cosb = cos_sb[:, sblk:sblk + 1, :].broadcast_to([128, HPC, DH])
            qf = pools["work"].tile([128, HPC, DH], f32, tag="qf", bufs=2)
            nc.vector.tensor_mul(qf, qn, cosb)
            qn4 = qn.rearrange("p h (t u) -> p h t u", u=32)
            sin4 = sin_sb[:, sblk:sblk + 1, :].rearrange(
                "p b (t u) -> p b t u", u=32)
            tmp = pools["work"].tile([128, HPC, 2, 32], f32, tag="tmp", bufs=2)
            nc.vector.tensor_mul(
                tmp[:, :, 0:1, :], qn4[:, :, 1:2, :],
                sin4[:, :, 0:1, :].broadcast_to([128, HPC, 1, 32]))
            nc.vector.tensor_mul(
                tmp[:, :, 1:2, :], qn4[:, :, 0:1, :],
                sin4[:, :, 1:2, :].broadcast_to([128, HPC, 1, 32]))
            qo = pools["work"].tile([128, E], bf16, tag="qo", bufs=3)
            nc.vector.tensor_add(
                qo, qf.rearrange("p h d -> p (h d)"),
                tmp.rearrange("p h t u -> p (h t u)"))
            return qo

        def emit_transposes(dstT, st, qos):
            for su, qo in enumerate(qos):
                sblk = st * 4 + su
                for et in range(ET):
                    trp = pools["tr"].tile([128, 128], bf16, tag="tr",
                                           name=f"tr{sblk}_{et}")
                    nc.tensor.transpose(trp, qo[:, et * 128:(et + 1) * 128],
                                        ident)
                    nc.vector.tensor_copy(
                        dstT[et][:, sblk * 128:(sblk + 1) * 128], trp)

        with tc.tile_pool(name="qkvw", bufs=1) as qkvw, \
             tc.tile_pool(name="xpool", bufs=2) as xpool, \
             tc.tile_pool(name="work", bufs=1) as work, \
             tc.tile_pool(name="pr_mm", bufs=6, space="PSUM") as mm_pool, \
             tc.tile_pool(name="pr_tr", bufs=2, space="PSUM") as tr_pool:
            pools["mm"] = mm_pool
            pools["tr"] = tr_pool
            pools["work"] = work

            # weights: wq first (first wave), wo into the persistent pool
            def load_w(dst, srcd, nchunk=8):
                fl = dst.rearrange("p t e -> p (t e)")
                tot = fl.shape[1]
                ch = tot // nchunk
                for c in range(nchunk):
                    nc.sync.dma_start(fl[:, c * ch:(c + 1) * ch],
                                      srcd[:, c * ch:(c + 1) * ch])

            wq_sb = qkvw.tile([128, DT, E], bf16, tag="wq")
            load_w(wq_sb, wqd)

            def load_x(st):
                xt = xpool.tile([128, DT, 512], bf16, tag="x", name=f"x{st}")
                fl = xt.rearrange("p t s -> p (t s)")
                base = st * DT * 512
                ch = DT * 512 // 16
                for c in range(16):
                    nc.sync.dma_start(fl[:, c * ch:(c + 1) * ch],
                                      xd[:, base + c * ch:base + (c + 1) * ch])
                return xt

            x_cur = load_x(0)
            wk_sb = qkvw.tile([128, DT, E], bf16, tag="wk")
            wv_sb = qkvw.tile([128, DT, E], bf16, tag="wv")
            wo_sb = wpool.tile([128, ET, DIM], bf16, tag="wo")
            load_w(wk_sb, wkd)
            load_w(wv_sb, wvd)
            load_w(wo_sb, wod)

            pending = None  # (dstT, st, qos) transposes deferred one wave
            for st in range(SB):
                x_nxt = load_x(st + 1) if st + 1 < SB else None
                for wname, w_sb_, dstT in ((f"q{st}", wq_sb, qT),
                                           (f"k{st}", wk_sb, kT),
                                           (f"v{st}", wv_sb, None)):
                    pss = emit_mms(w_sb_, x_cur, wname)
                    if dstT is None:
                        for su in range(4):
                            eng = nc.scalar.copy if su % 2 == 0 \
                                else nc.vector.tensor_copy
                            eng(v_sb[:, st * 4 + su, :, 0:64],
                                pss[su].rearrange("p (h d) -> p h d", d=DH))
                        qos = None
                    else:
                        # release psum banks first (Pool is in-order; the
                        # chains below stall it on ACT results)
                        ps_ss = []
                        for su in range(4):
                            ps_s = pools["work"].tile(
                                [128, E], f32, tag="pss",
                                bufs=4, name=f"pss_{wname}_{su}")
                            if su % 2 == 0:
                                nc.scalar.copy(ps_s, pss[su])
                            else:
                                nc.vector.tensor_copy(ps_s, pss[su])
                            ps_ss.append(ps_s)
                        qos = [emit_chain_qk(ps_ss[su], st, su)
                               for su in range(4)]
                    if pending is not None:
                        emit_transposes(*pending)
                        pending = None
                    if qos is not None:
                        pending = (dstT, st, qos)
                x_cur = x_nxt
            if pending is not None:
                emit_transposes(*pending)

        # ---------------- attention + output projection ----------------
        def attn_head(h, i, pv):
            """head h, si-512 block i; accumulates into held pv psum.
            row 64 of pv collects softmax denominators (ones column in v)."""
            et, hp = h // 2, (h % 2) * 64
            npr = 2 * (i + 1)
            lgs = {}

            def emit_lg(p):
                lg2 = pools["lg"].tile([128, 2, 512], f32, tag="lg",
                                       name=f"lg{i}_{h}_{p}")
                for bb in range(2):
                    sjb = 2 * p + bb
                    nc.tensor.matmul(
                        lg2[:, bb, :],
                        kT[et][hp:hp + 64, sjb * 128:(sjb + 1) * 128],
                        qT[et][hp:hp + 64, i * 512:(i + 1) * 512],
                        start=True, stop=True)
                lgs[p] = lg2

            emit_lg(0)
            if npr > 1:
                emit_lg(1)
            for p in range(npr):
                lg2 = lgs.pop(p)
                ex = pools["attnp"].tile([128, 2, 512], bf16, tag="ex", bufs=3)
                r0 = 2 * p - 4 * i  # diag offset of the pair's first block
                c0p = max(0, r0) * 128
                if c0p > 0:
                    nc.scalar.activation(ex[:, :, c0p:], lg2[:, :, c0p:],
                                         AF.Exp)
                else:
                    nc.scalar.activation(ex, lg2, AF.Exp)
                if p + 2 < npr:
                    emit_lg(p + 2)
                for bb in range(2):
                    sjb = 2 * p + bb
                    r = sjb - 4 * i
                    if r >= 0:
                        # only the [c0p, (r+1)*128) range holds the causal
                        # boundary (plus below-diagonal cols of the pair's
                        # second block); the rest is either fully valid or
                        # excluded from the pv matmul below
                        ce = (r + 1) * 128
                        nc.gpsimd.tensor_mul(ex[:, bb, c0p:ce],
                                             ex[:, bb, c0p:ce],
                                             mask_sb[:, r, c0p:ce])
                    cpv = max(0, r) * 128
                    nc.tensor.matmul(
                        pv[0:65, cpv:],
                        v_sb[:, sjb, h, :],
                        ex[:, bb, cpv:],
                        start=(sjb == 0), stop=(sjb == 4 * i + 3),
                        skip_group_check=True)

        def normalize_pair(pvs_list, inv8b, i, j):
            """divide heads 2j,2j+1 of block i by softmax denominators."""
            bc = pools["yb"].tile([128, 512], f32, tag="yb",
                                  name=f"bc{i}_{j}")
            nc.tensor.matmul(bc, sel2, inv8b[:, j, :],
                             start=True, stop=True)
            for kk in range(2):
                h = 2 * j + kk
                et, hp = h // 2, (h % 2) * 64
                nc.vector.tensor_mul(
                    qT[et][hp:hp + 64, i * 512:(i + 1) * 512],
                    pvs_list[h][0:64, :], bc[kk * 64:(kk + 1) * 64, :])

        def yproj(ib):
            """si-128 block ib -> Y[ib*128:+128, :]."""
            for nd in range(4):
                ps = pools["yb"].tile([128, 512], f32, tag="yb",
                                      name=f"yps{ib}_{nd}")
                for ket in range(ET):
                    nc.tensor.matmul(
                        ps,
                        qT[ket][:, ib * 128:(ib + 1) * 128],
                        wo_sb[:, ket, nd * 512:(nd + 1) * 512],
                        start=(ket == 0), stop=(ket == ET - 1))
                ys = pools["ypool"].tile([128, 512], f32, tag="y", bufs=4)
                if nd % 2 == 0:
                    nc.scalar.copy(ys, ps)
                else:
                    nc.vector.tensor_copy(ys, ps)
                nc.sync.dma_start(
                    Y[ib * 128:(ib + 1) * 128, nd * 512:(nd + 1) * 512], ys)

        with tc.tile_pool(name="attnp", bufs=1) as attnp, \
             tc.tile_pool(name="ypool", bufs=1) as ypool, \
             tc.tile_pool(name="ps_lg", bufs=2, space="PSUM") as lg_pool, \
             tc.tile_pool(name="ps_pv", bufs=2, space="PSUM") as pv_pool, \
             tc.tile_pool(name="ps_yb", bufs=2, space="PSUM") as yb_pool:
            pools["lg"] = lg_pool
            pools["pv"] = pv_pool
            pools["yb"] = yb_pool
            pools["attnp"] = attnp
            pools["ypool"] = ypool
            prev = None  # (i, pvs_list, inv8b) of previous block
            dn8 = pools["attnp"].tile([33, 4, 512], f32, tag="dn8", bufs=1,
                                      name="dn8")
            nc.vector.memset(dn8, 1.0)  # rows 1..31 stay 1.0 (recip-safe)
            for i in range(SB):
                cur_pvs = []
                for h in range(HPC):
                    pv = pv_pool.tile([128, 512], f32, tag="pv",
                                      name=f"pv{i}_{h}")
                    attn_head(h, i, pv)
                    pvs = pools["attnp"].tile([65, 512], f32, tag="pvs", bufs=12,
                                     name=f"pvs{i}_{h}")
                    nc.vector.tensor_copy(pvs, pv[0:65, :])  # frees psum
                    pr = (h % 2) * 32
                    nc.scalar.copy(dn8[pr:pr + 1, h // 2, :], pvs[64:65, :])
                    cur_pvs.append(pvs)
                    if h % 2 == 1 and prev is not None:
                        j = h // 2
                        pi, ppvs, pinv = prev
                        normalize_pair(ppvs, pinv, pi, j)
                        if pi > 0:
                            yproj(4 * (pi - 1) + j)
                inv8 = pools["attnp"].tile([33, 4, 512], f32, tag="inv8", bufs=1,
                                  name=f"inv8_{i}")
                nc.vector.reciprocal_approx_fast(inv8, dn8)
                inv8b = pools["attnp"].tile([33, 4, 512], bf16, tag="inv8b", bufs=2,
                                   name=f"inv8b_{i}")
                nc.scalar.copy(inv8b, inv8)
                prev = (i, cur_pvs, inv8b)
            # flush: yproj block 2 first (fills PE while the last
            # denominators come through recip/cast), then block 3
            pi, ppvs, pinv = prev
            for j in range(4):
                yproj(4 * (pi - 1) + j)
                normalize_pair(ppvs, pinv, pi, j)
            for ib in range(4 * pi, 4 * pi + 4):
                yproj(ib)

    return nc


def _host_prep(x, wq, wk, wv, wo, qk_scale):
    """Returns per-core input dicts."""
    perm = np.concatenate([np.arange(0, DH, 2), np.arange(1, DH, 2)])
    wq_n = _l2n(wq, -1).reshape(HEADS, DH, DIM)[:, perm, :].reshape(HEADS * DH, DIM)
    wk_n = _l2n(wk, -1).reshape(HEADS, DH, DIM)[:, perm, :].reshape(HEADS * DH, DIM)
    wv_n = _l2n(wv, -1)
    wo_n = _l2n(wo, 0)
    sp = qk_scale.astype(np.float64)[perm]

    # rope tables with qk_scale folded in; permuted-block layout
    half = np.arange(0, DH, 2)
    freqs = 1.0 / (THETA ** (half.astype(np.float64) / DH))      # (32,)
    ang = np.arange(S, dtype=np.float64)[:, None] * freqs[None]  # (S, 32)
    cos_h, sin_h = np.cos(ang), np.sin(ang)
    cos_p = np.concatenate([cos_h, cos_h], 1)                    # (S, 64)
    sin_e = np.concatenate([-sin_h, sin_h], 1)
    cos_eff = (cos_p * sp[None, :]).astype(np.float32)
    swap_sp = np.concatenate([sp[32:], sp[:32]])
    sin_eff = (sin_e * swap_sp[None, :]).astype(np.float32)
    # device layout [128, SS*DH]: [p, b*64+c] = tbl[b*128+p, c]
    cosd = np.ascontiguousarray(
        cos_eff.reshape(SS, 128, DH).transpose(1, 0, 2).reshape(128, SS * DH))
    sind = np.ascontiguousarray(
        sin_eff.reshape(SS, 128, DH).transpose(1, 0, 2).reshape(128, SS * DH))

    # causal masks for the 4 diagonal offsets: keep sjl + 128r <= sil
    sjl = np.arange(128)[:, None]
    sil = np.arange(512)[None, :]
    maskd = np.ascontiguousarray(np.concatenate(
        [(sjl + 128 * r <= sil).astype(np.float32) for r in range(4)],
        axis=1))  # [128, 4*512]

    def wlay(a):  # [E, DIM] -> [128, DT*E] with [p, t*E+e] = a[e, t*128+p]
        return np.ascontiguousarray(
            a.T.reshape(DT, 128, E).transpose(1, 0, 2).reshape(128, DT * E))

    sel2h = np.zeros((33, 128), dtype=np.float32)
    sel2h[0, 0:64] = 1.0
    sel2h[32, 64:128] = 1.0

    in_maps = []
    for c in range(NCORES):
        b, t = divmod(c, TP)
        e0 = t * E
        xb = x[b]  # [S, DIM]
        xdev = np.ascontiguousarray(
            xb.reshape(SB, 512, DT, 128).transpose(3, 0, 2, 1)
            .reshape(128, SB * DT * 512))
        wob = wo_n[:, e0:e0 + E]  # [DIM, E]
        wodev = np.ascontiguousarray(
            wob.T.reshape(ET, 128, DIM).transpose(1, 0, 2)
            .reshape(128, ET * DIM))
        in_maps.append({
            "xd": xdev.astype(BF16),
            "wqd": wlay(wq_n[e0:e0 + E]).astype(BF16),
            "wkd": wlay(wk_n[e0:e0 + E]).astype(BF16),
            "wvd": wlay(wv_n[e0:e0 + E]).astype(BF16),
            "wod": wodev.astype(BF16),
            "cosd": cosd.astype(BF16), "sind": sind.astype(BF16),
            "seld": sel2h.astype(BF16),
            "maskd": maskd.astype(BF16),
        })
    return in_maps


def _install_profile_hook():
    """antenv.axon_hooks is absent in this image; shim it and register the
    ctypes NTFF hook against /opt/axon/libaxon_pjrt.so (mirrors trn_boot)."""
    import types
    import ctypes
    import contextlib

    try:
        from antenv.axon_hooks import get_axon_ntff_profile_hook  # noqa
        return
    except ImportError:
        pass
    import antenv
    mod = types.ModuleType("antenv.axon_hooks")
    state = {}
    mod.set_axon_ntff_profile_hook = lambda h: state.__setitem__("h", h)
    mod.get_axon_ntff_profile_hook = lambda: state.get("h")
    sys.modules["antenv.axon_hooks"] = mod
    antenv.axon_hooks = mod

    so_path = "/opt/axon/libaxon_pjrt.so"
    lib = ctypes.CDLL(so_path)
    if not hasattr(lib, "axon_start_nrt_profile"):
        return
    lib.axon_start_nrt_profile.argtypes = [
        ctypes.POINTER(ctypes.c_int64), ctypes.c_size_t]
    lib.axon_start_nrt_profile.restype = ctypes.c_int64
    lib.axon_stop_nrt_profile.argtypes = [ctypes.c_char_p]
    lib.axon_stop_nrt_profile.restype = ctypes.c_int64

    @contextlib.contextmanager
    def _hook(output_dir, device_ids):
        import jax
        jax.devices()
        if device_ids:
            ids = (ctypes.c_int64 * len(device_ids))(*device_ids)
            rc = lib.axon_start_nrt_profile(ids, len(device_ids))
        else:
            rc = lib.axon_start_nrt_profile(None, 0)
        if rc != 0:
            raise RuntimeError(f"axon_start_nrt_profile rc={rc}")
        try:
            yield
        finally:
            n = lib.axon_stop_nrt_profile(str(output_dir).encode())
            print(f"profile: {n} file(s) written to {output_dir}",
                  file=sys.stderr)

    mod.set_axon_ntff_profile_hook(_hook)


def kernel(x, wq, wk, wv, wo, qk_scale, _profile=False):
    from concourse.bass_utils import run_bass_kernel_spmd

    if _profile:
        _install_profile_hook()

    if "nc" not in _CACHE:
        nc = _build_program()
        nc.finalize()
        _CACHE["nc"] = nc
    nc = _CACHE["nc"]
    in_maps = _host_prep(np.asarray(x), np.asarray(wq), np.asarray(wk),
                         np.asarray(wv), np.asarray(wo), np.asarray(qk_scale))
    res = run_bass_kernel_spmd(nc, in_maps, core_ids=list(range(NCORES)),
                               trace=_profile)
    outs = res.results
    y = np.empty((B, S, DIM), dtype=np.float32)
    for b in range(B):
        y[b] = sum(outs[b * TP + t]["Y"] for t in range(TP))
    if _profile:
        _CACHE["last_exec_time_ns"] = res.exec_time_ns
        _CACHE["last_profile"] = res.profile_json
    return y


# revision 20
# speedup vs baseline: 1.2194x; 1.0039x over previous
"""nn_Attention Trainium2 Bass kernel (optimized).

Full attention forward: x->(q,k,v) with l2-normalized weights, per-head-dim
l2 norm + learned qk scale, interleaved RoPE, causal SDPA, output projection
with column-l2-normalized wo.

Sharding: TP=4 over heads (8 heads/core) x DP=2 over batch across 8 cores.
Each core computes a partial [2048, 2048] output for its batch; host sums
the 4 TP partials per batch.

Key optimizations over the first working version:
- host-side per-partition-contiguous DRAM layouts (64KB descriptors);
  x streamed per 512-row block so the first projection wave starts after
  ~2MB of DMA instead of 16MB.
- projection waves software-pipelined: psum released early via a Pool-engine
  copy; the qT/kT transposes of wave w are emitted after wave w+1's matmuls
  so the PE never waits on the vector norm/rope chain (keeps the PE p-state
  ramped at 2.4 GHz).
- rsqrt via reciprocal_approx_fast (+scalar sqrt) instead of the slow DVE
  reciprocal; rope rotate-half via strided half-views (no rot copies).
- attention: pv psum copied to SBUF by the Pool engine immediately
  (psum ring of 2), denominators batched per block ([8,512] fast recip),
  1/denom broadcast via one packed bf16 matmul per head pair, normalize +
  output projection software-pipelined across blocks (block i+1 attention |
  block i normalize | block i-1 yproj) so the PE stream never breaks.
- mask multiplies, psum->sbuf copies on the otherwise-idle Pool engine.
"""
import sys
import os
import math
from contextlib import ExitStack

sys.path.insert(0, "/opt/trn_rl_repo")

import numpy as np
import ml_dtypes

BF16 = ml_dtypes.bfloat16

B, S, DIM = 2, 2048, 2048
HEADS, DH = 32, 64
THETA = 10000.0
NCORES = 8
TP = 4             # head-parallel ways
HPC = HEADS // TP  # heads per core = 8
E = HPC * DH       # per-core qkv width = 512
ET = E // 128      # e-tiles per core = 4
DT = DIM // 128    # contraction d-tiles = 16
SB = S // 512      # 512-wide seq blocks = 4
SS = S // 128      # 128-wide seq blocks = 16

_CACHE = {}


def _l2n(w, axis):
    n = np.sqrt((w.astype(np.float64) ** 2).sum(axis=axis, keepdims=True))
    n = np.maximum(n, 1e-12)
    return (w / n).astype(np.float32)


def _build_program():
    import concourse.bass as bass
    from concourse import bacc
    import concourse.mybir as mybir
    import concourse.tile as tile
    from concourse.masks import make_identity

    f32 = mybir.dt.float32
    bf16 = mybir.dt.bfloat16
    AF = mybir.ActivationFunctionType
    AX = mybir.AxisListType
    OP = mybir.AluOpType

    nc = bacc.Bacc("TRN2", target_bir_lowering=False)

    xd = nc.dram_tensor("xd", [128, SB * DT * 512], bf16, kind="ExternalInput")
    wqd = nc.dram_tensor("wqd", [128, DT * E], bf16, kind="ExternalInput")
    wkd = nc.dram_tensor("wkd", [128, DT * E], bf16, kind="ExternalInput")
    wvd = nc.dram_tensor("wvd", [128, DT * E], bf16, kind="ExternalInput")
    wod = nc.dram_tensor("wod", [128, ET * DIM], bf16, kind="ExternalInput")
    cosd = nc.dram_tensor("cosd", [128, SS * DH], bf16, kind="ExternalInput")
    sind = nc.dram_tensor("sind", [128, SS * DH], bf16, kind="ExternalInput")
    maskd = nc.dram_tensor("maskd", [128, 4 * 512], bf16, kind="ExternalInput")
    seld = nc.dram_tensor("seld", [33, 128], bf16, kind="ExternalInput")
    Y = nc.dram_tensor("Y", [S, DIM], f32, kind="ExternalOutput")

    with tile.TileContext(nc) as tc, ExitStack() as ctx:
        const = ctx.enter_context(tc.tile_pool(name="const", bufs=1))
        wpool = ctx.enter_context(tc.tile_pool(name="wpool", bufs=1))
        qkv = ctx.enter_context(tc.tile_pool(name="qkv", bufs=1))

        # constants (loaded first; small)
        cos_sb = const.tile([128, SS, DH], bf16)
        sin_sb = const.tile([128, SS, DH], bf16)
        mask_sb = const.tile([128, 4, 512], bf16)
        nc.sync.dma_start(cos_sb, cosd.rearrange("p (b d) -> p b d", d=DH))
        nc.sync.dma_start(sin_sb, sind.rearrange("p (b d) -> p b d", d=DH))
        nc.sync.dma_start(mask_sb, maskd.rearrange("p (r n) -> p r n", n=512))
        ident = const.tile([128, 128], bf16)
        make_identity(nc, ident)
        # head-pair selector for the 1/denom broadcast matmul:
        # sel2[0, 0:64]=1, sel2[1, 64:128]=1
        sel2 = const.tile([33, 128], bf16)
        nc.sync.dma_start(sel2, seld[:, :])


        # persistent activations
        qT = [qkv.tile([128, S], bf16, tag=f"qT{e}", name=f"qT{e}")
              for e in range(ET)]
        kT = [qkv.tile([128, S], bf16, tag=f"kT{e}", name=f"kT{e}")
              for e in range(ET)]
        v_sb = qkv.tile([128, SS, HPC, 65], bf16, tag="v")
        nc.vector.memset(v_sb[:, :, :, 64:65], 1.0)

        # ---------------- projection phase ----------------
        pools = {}

        def emit_mms(w_sb_, x_t, wname):
            pss = [pools["mm"].tile([128, E], f32, tag="mm",
                                    name=f"ps_{wname}_{su}")
                   for su in range(4)]
            for dt in range(DT):
                for su in range(4):
                    nc.tensor.matmul(
                        pss[su], x_t[:, dt, su * 128:(su + 1) * 128],
                        w_sb_[:, dt, :],
                        start=(dt == 0), stop=(dt == DT - 1))
            return pss

        def emit_chain_qk(ps_s, st, su):
            """norm + rope for one si-128 block; returns bf16 qo tile."""
            sblk = st * 4 + su
            sq = pools["work"].tile([128, E], f32, tag="sq", bufs=2)
            nc.scalar.square(sq, ps_s)
            ssq = pools["work"].tile([128, HPC], f32, tag="ssq", bufs=2)
            nc.vector.tensor_reduce(
                ssq, sq.rearrange("p (h d) -> p h d", d=DH),
                axis=AX.X, op=OP.add)
            rr = pools["work"].tile([128, HPC], f32, tag="rr", bufs=2)
            nc.vector.reciprocal_approx_fast(rr, ssq)
            inv = pools["work"].tile([128, HPC], f32, tag="inv", bufs=2)
            nc.scalar.sqrt(inv, rr)  # 1/|q| per (row, head)
            qn = pools["work"].tile([128, HPC, DH], f32, tag="qn", bufs=2)
            nc.gpsimd.tensor_mul(
                qn, ps_s.rearrange("p (h d) -> p h d", d=DH),
                inv.unsqueeze(2).broadcast_to([128, HPC, DH]))
            # rope: q' = qn*cos + swap32(qn)*sin  (sign folded into sin table)
            cosb = cos_sb[:, sblk:sblk + 1, :].broadcast_to([128, HPC, DH])
            qf = pools["work"].tile([128, HPC, DH], f32, tag="qf", bufs=2)
            nc.vector.tensor_mul(qf, qn, cosb)
            qn4 = qn.rearrange("p h (t u) -> p h t u", u=32)
            sin4 = sin_sb[:, sblk:sblk + 1, :].rearrange(
                "p b (t u) -> p b t u", u=32)
            tmp = pools["work"].tile([128, HPC, 2, 32], f32, tag="tmp", bufs=2)
            nc.vector.tensor_mul(
                tmp[:, :, 0:1, :], qn4[:, :, 1:2, :],
                sin4[:, :, 0:1, :].broadcast_to([128, HPC, 1, 32]))
            nc.vector.tensor_mul(
                tmp[:, :, 1:2, :], qn4[:, :, 0:1, :],
                sin4[:, :, 1:2, :].broadcast_to([128, HPC, 1, 32]))
            qo = pools["work"].tile([128, E], bf16, tag="qo", bufs=3)
            nc.vector.tensor_add(
                qo, qf.rearrange("p h d -> p (h d)"),
                tmp.rearrange("p h t u -> p (h t u)"))
            return qo

        def emit_transposes(dstT, st, qos):
            for su, qo in enumerate(qos):
                sblk = st * 4 + su
                for et in range(ET):
                    trp = pools["tr"].tile([128, 128], bf16, tag="tr",
                                           name=f"tr{sblk}_{et}")
                    nc.tensor.transpose(trp, qo[:, et * 128:(et + 1) * 128],
                                        ident)
                    nc.vector.tensor_copy(
                        dstT[et][:, sblk * 128:(sblk + 1) * 128], trp)

        with tc.tile_pool(name="qkvw", bufs=1) as qkvw, \
             tc.tile_pool(name="xpool", bufs=2) as xpool, \
             tc.tile_pool(name="work", bufs=1) as work, \
             tc.tile_pool(name="pr_mm", bufs=6, space="PSUM") as mm_pool, \
             tc.tile_pool(name="pr_tr", bufs=2, space="PSUM") as tr_pool:
            pools["mm"] = mm_pool
            pools["tr"] = tr_pool
            pools["work"] = work

            # weights: wq first (first wave), wo into the persistent pool
            def load_w(dst, srcd, nchunk=8):
                fl = dst.rearrange("p t e -> p (t e)")
                tot = fl.shape[1]
                ch = tot // nchunk
                for c in range(nchunk):
                    nc.sync.dma_start(fl[:, c * ch:(c + 1) * ch],
                                      srcd[:, c * ch:(c + 1) * ch])

            wq_sb = qkvw.tile([128, DT, E], bf16, tag="wq")
            load_w(wq_sb, wqd)

            def load_x(st):
                xt = xpool.tile([128, DT, 512], bf16, tag="x", name=f"x{st}")
                fl = xt.rearrange("p t s -> p (t s)")
                base = st * DT * 512
                ch = DT * 512 // 16
                for c in range(16):
                    nc.sync.dma_start(fl[:, c * ch:(c + 1) * ch],
                                      xd[:, base + c * ch:base + (c + 1) * ch])
                return xt

            x_cur = load_x(0)
            wk_sb = qkvw.tile([128, DT, E], bf16, tag="wk")
            wv_sb = qkvw.tile([128, DT, E], bf16, tag="wv")
            wo_sb = wpool.tile([128, ET, DIM], bf16, tag="wo")
            load_w(wk_sb, wkd)
            load_w(wv_sb, wvd)
            load_w(wo_sb, wod)

            pending = None  # (dstT, st, qos) transposes deferred one wave
            for st in range(SB):
                x_nxt = load_x(st + 1) if st + 1 < SB else None
                for wname, w_sb_, dstT in ((f"q{st}", wq_sb, qT),
                                           (f"k{st}", wk_sb, kT),
                                           (f"v{st}", wv_sb, None)):
                    pss = emit_mms(w_sb_, x_cur, wname)
                    if dstT is None:
                        for su in range(4):
                            eng = nc.scalar.copy if su % 2 == 0 \
                                else nc.vector.tensor_copy
                            eng(v_sb[:, st * 4 + su, :, 0:64],
                                pss[su].rearrange("p (h d) -> p h d", d=DH))
                        qos = None
                    else:
                        # release psum banks first (Pool is in-order; the
                        # chains below stall it on ACT results)
                        ps_ss = []
                        for su in range(4):
                            ps_s = pools["work"].tile(
                                [128, E], f32, tag="pss",
                                bufs=4, name=f"pss_{wname}_{su}")
                            if su % 2 == 0:
                                nc.scalar.copy(ps_s, pss[su])
                            else:
                                nc.vector.tensor_copy(ps_s, pss[su])
                            ps_ss.append(ps_s)
                        qos = [emit_chain_qk(ps_ss[su], st, su)
                               for su in range(4)]
                    if pending is not None:
                        emit_transposes(*pending)
                        pending = None
                    if qos is not None:
                        pending = (dstT, st, qos)
                x_cur = x_nxt
            if pending is not None:
                emit_transposes(*pending)

        # ---------------- attention + output projection ----------------
        def emit_lg(i, h, p, pvh):
            """logit pair matmuls for (head h, si-block i, pair p)."""
            et, hp = h // 2, (h % 2) * 64
            lg2 = pools["lg"].tile([128, 2, 512], f32, tag="lg",
                                   name=f"lg{i}_{h}_{p}")
            for bb in range(2):
                sjb = 2 * p + bb
                nc.tensor.matmul(
                    lg2[:, bb, :],
                    kT[et][hp:hp + 64, sjb * 128:(sjb + 1) * 128],
                    qT[et][hp:hp + 64, i * 512:(i + 1) * 512],
                    start=True, stop=True)
            return lg2

        def pair_step(i, h, p, lg2, pvh):
            """exp + mask + pv accumulation for one (h, pair) step."""
            ex = pools["attnp"].tile([128, 2, 512], bf16, tag="ex", bufs=4)
            r0 = 2 * p - 4 * i  # diag offset of the pair's first block
            c0p = max(0, r0) * 128
            if c0p > 0:
                nc.scalar.activation(ex[:, :, c0p:], lg2[:, :, c0p:], AF.Exp)
            else:
                nc.scalar.activation(ex, lg2, AF.Exp)
            return ex

        def pair_pv(i, h, p, ex, pvh):
            for bb in range(2):
                sjb = 2 * p + bb
                r = sjb - 4 * i
                r0 = 2 * p - 4 * i
                c0p = max(0, r0) * 128
                if r >= 0:
                    ce = (r + 1) * 128
                    nc.gpsimd.tensor_mul(ex[:, bb, c0p:ce],
                                         ex[:, bb, c0p:ce],
                                         mask_sb[:, r, c0p:ce])
                cpv = max(0, r) * 128
                nc.tensor.matmul(
                    pvh[0:65, cpv:],
                    v_sb[:, sjb, h, :],
                    ex[:, bb, cpv:],
                    start=(sjb == 0), stop=(sjb == 4 * i + 3),
                    skip_group_check=True)

        def normalize_pair(pvs_list, inv8b, i, j):
            """divide heads 2j,2j+1 of block i by softmax denominators."""
            bc = pools["yb"].tile([128, 512], f32, tag="yb",
                                  name=f"bc{i}_{j}")
            nc.tensor.matmul(bc, sel2, inv8b[:, j, :],
                             start=True, stop=True)
            for kk in range(2):
                h = 2 * j + kk
                et, hp = h // 2, (h % 2) * 64
                nc.vector.tensor_mul(
                    qT[et][hp:hp + 64, i * 512:(i + 1) * 512],
                    pvs_list[h][0:64, :], bc[kk * 64:(kk + 1) * 64, :])

        def yproj(ib):
            """si-128 block ib -> Y[ib*128:+128, :]."""
            for nd in range(4):
                ps = pools["yb"].tile([128, 512], f32, tag="yb",
                                      name=f"yps{ib}_{nd}")
                for ket in range(ET):
                    nc.tensor.matmul(
                        ps,
                        qT[ket][:, ib * 128:(ib + 1) * 128],
                        wo_sb[:, ket, nd * 512:(nd + 1) * 512],
                        start=(ket == 0), stop=(ket == ET - 1))
                ys = pools["ypool"].tile([128, 512], f32, tag="y", bufs=4)
                if nd % 2 == 0:
                    nc.scalar.copy(ys, ps)
                else:
                    nc.vector.tensor_copy(ys, ps)
                nc.sync.dma_start(
                    Y[ib * 128:(ib + 1) * 128, nd * 512:(nd + 1) * 512], ys)

        with tc.tile_pool(name="attnp", bufs=1) as attnp, \
             tc.tile_pool(name="ypool", bufs=1) as ypool, \
             tc.tile_pool(name="ps_lg", bufs=2, space="PSUM") as lg_pool, \
             tc.tile_pool(name="ps_pv", bufs=2, space="PSUM") as pv_pool, \
             tc.tile_pool(name="ps_yb", bufs=2, space="PSUM") as yb_pool:
            pools["lg"] = lg_pool
            pools["pv"] = pv_pool
            pools["yb"] = yb_pool
            pools["attnp"] = attnp
            pools["ypool"] = ypool
            prev = None  # (i, pvs_list, inv8b) of previous block
            dn8 = pools["attnp"].tile([33, 4, 512], f32, tag="dn8", bufs=1,
                                      name="dn8")
            nc.vector.memset(dn8, 1.0)  # rows 1..31 stay 1.0 (recip-safe)
            for i in range(SB):
                npr = 2 * (i + 1)
                steps = [(h, p) for h in range(HPC) for p in range(npr)]
                pvt = {}
                for h in range(HPC):
                    pvt[h] = None  # allocated lazily at first lg emission
                lgt = {}
                exd = {}

                def get_pv(h):
                    if pvt[h] is None:
                        pvt[h] = pv_pool.tile([128, 512], f32, tag="pv",
                                              name=f"pv{i}_{h}")
                    return pvt[h]

                # prime lookahead of 2 pair-steps
                for k in range(min(2, len(steps))):
                    h, p = steps[k]
                    lgt[(h, p)] = emit_lg(i, h, p, get_pv(h))
                cur_pvs = []
                for k, (h, p) in enumerate(steps):
                    lg2 = lgt.pop((h, p))
                    ex = pair_step(i, h, p, lg2, get_pv(h))
                    if k + 2 < len(steps):
                        h2, p2 = steps[k + 2]
                        lgt[(h2, p2)] = emit_lg(i, h2, p2, get_pv(h2))
                    pair_pv(i, h, p, ex, get_pv(h))
                    if p == npr - 1:
                        # head h complete: drain pv psum, denominators
                        pv = pvt[h]
                        pvs = pools["attnp"].tile([65, 512], f32, tag="pvs",
                                                  bufs=12, name=f"pvs{i}_{h}")
                        nc.vector.tensor_copy(pvs, pv[0:65, :])  # frees psum
                        pr = (h % 2) * 32
                        nc.scalar.copy(dn8[pr:pr + 1, h // 2, :],
                                       pv[64:65, :])
                        cur_pvs.append(pvs)
                        if h % 2 == 1 and prev is not None:
                            j = h // 2
                            pi, ppvs, pinv = prev
                            normalize_pair(ppvs, pinv, pi, j)
                            if pi > 0:
                                yproj(4 * (pi - 1) + j)
                inv8 = pools["attnp"].tile([33, 4, 512], f32, tag="inv8",
                                           bufs=1, name=f"inv8_{i}")
                nc.vector.reciprocal_approx_fast(inv8, dn8)
                inv8b = pools["attnp"].tile([33, 4, 512], bf16, tag="inv8b",
                                            bufs=2, name=f"inv8b_{i}")
                nc.scalar.copy(inv8b, inv8)
                prev = (i, cur_pvs, inv8b)
            # flush: yproj block 2 first (fills PE while the last
            # denominators come through recip/cast), then block 3
            pi, ppvs, pinv = prev
            for j in range(4):
                yproj(4 * (pi - 1) + j)
                normalize_pair(ppvs, pinv, pi, j)
            for ib in range(4 * pi, 4 * pi + 4):
                yproj(ib)

    return nc


def _host_prep(x, wq, wk, wv, wo, qk_scale):
    """Returns per-core input dicts."""
    perm = np.concatenate([np.arange(0, DH, 2), np.arange(1, DH, 2)])
    wq_n = _l2n(wq, -1).reshape(HEADS, DH, DIM)[:, perm, :].reshape(HEADS * DH, DIM)
    wk_n = _l2n(wk, -1).reshape(HEADS, DH, DIM)[:, perm, :].reshape(HEADS * DH, DIM)
    wv_n = _l2n(wv, -1)
    wo_n = _l2n(wo, 0)
    sp = qk_scale.astype(np.float64)[perm]

    # rope tables with qk_scale folded in; permuted-block layout
    half = np.arange(0, DH, 2)
    freqs = 1.0 / (THETA ** (half.astype(np.float64) / DH))      # (32,)
    ang = np.arange(S, dtype=np.float64)[:, None] * freqs[None]  # (S, 32)
    cos_h, sin_h = np.cos(ang), np.sin(ang)
    cos_p = np.concatenate([cos_h, cos_h], 1)                    # (S, 64)
    sin_e = np.concatenate([-sin_h, sin_h], 1)
    cos_eff = (cos_p * sp[None, :]).astype(np.float32)
    swap_sp = np.concatenate([sp[32:], sp[:32]])
    sin_eff = (sin_e * swap_sp[None, :]).astype(np.float32)
    # device layout [128, SS*DH]: [p, b*64+c] = tbl[b*128+p, c]
    cosd = np.ascontiguousarray(
        cos_eff.reshape(SS, 128, DH).transpose(1, 0, 2).reshape(128, SS * DH))
    sind = np.ascontiguousarray(
        sin_eff.reshape(SS, 128, DH).transpose(1, 0, 2).reshape(128, SS * DH))

    # causal masks for the 4 diagonal offsets: keep sjl + 128r <= sil
    sjl = np.arange(128)[:, None]
    sil = np.arange(512)[None, :]
    maskd = np.ascontiguousarray(np.concatenate(
        [(sjl + 128 * r <= sil).astype(np.float32) for r in range(4)],
        axis=1))  # [128, 4*512]

    def wlay(a):  # [E, DIM] -> [128, DT*E] with [p, t*E+e] = a[e, t*128+p]
        return np.ascontiguousarray(
            a.T.reshape(DT, 128, E).transpose(1, 0, 2).reshape(128, DT * E))

    sel2h = np.zeros((33, 128), dtype=np.float32)
    sel2h[0, 0:64] = 1.0
    sel2h[32, 64:128] = 1.0

    in_maps = []
    for c in range(NCORES):
        b, t = divmod(c, TP)
        e0 = t * E
        xb = x[b]  # [S, DIM]
        xdev = np.ascontiguousarray(
            xb.reshape(SB, 512, DT, 128).transpose(3, 0, 2, 1)
            .reshape(128, SB * DT * 512))
        wob = wo_n[:, e0:e0 + E]  # [DIM, E]
        wodev = np.ascontiguousarray(
            wob.T.reshape(ET, 128, DIM).transpose(1, 0, 2)
            .reshape(128, ET * DIM))
        in_maps.append({
            "xd": xdev.astype(BF16),
            "wqd": wlay(wq_n[e0:e0 + E]).astype(BF16),
            "wkd": wlay(wk_n[e0:e0 + E]).astype(BF16),
            "wvd": wlay(wv_n[e0:e0 + E]).astype(BF16),
            "wod": wodev.astype(BF16),
            "cosd": cosd.astype(BF16), "sind": sind.astype(BF16),
            "seld": sel2h.astype(BF16),
            "maskd": maskd.astype(BF16),
        })
    return in_maps


def _install_profile_hook():
    """antenv.axon_hooks is absent in this image; shim it and register the
    ctypes NTFF hook against /opt/axon/libaxon_pjrt.so (mirrors trn_boot)."""
    import types
    import ctypes
    import contextlib

    try:
        from antenv.axon_hooks import get_axon_ntff_profile_hook  # noqa
        return
    except ImportError:
        pass
    import antenv
    mod = types.ModuleType("antenv.axon_hooks")
    state = {}
    mod.set_axon_ntff_profile_hook = lambda h: state.__setitem__("h", h)
    mod.get_axon_ntff_profile_hook = lambda: state.get("h")
    sys.modules["antenv.axon_hooks"] = mod
    antenv.axon_hooks = mod

    so_path = "/opt/axon/libaxon_pjrt.so"
    lib = ctypes.CDLL(so_path)
    if not hasattr(lib, "axon_start_nrt_profile"):
        return
    lib.axon_start_nrt_profile.argtypes = [
        ctypes.POINTER(ctypes.c_int64), ctypes.c_size_t]
    lib.axon_start_nrt_profile.restype = ctypes.c_int64
    lib.axon_stop_nrt_profile.argtypes = [ctypes.c_char_p]
    lib.axon_stop_nrt_profile.restype = ctypes.c_int64

    @contextlib.contextmanager
    def _hook(output_dir, device_ids):
        import jax
        jax.devices()
        if device_ids:
            ids = (ctypes.c_int64 * len(device_ids))(*device_ids)
            rc = lib.axon_start_nrt_profile(ids, len(device_ids))
        else:
            rc = lib.axon_start_nrt_profile(None, 0)
        if rc != 0:
            raise RuntimeError(f"axon_start_nrt_profile rc={rc}")
        try:
            yield
        finally:
            n = lib.axon_stop_nrt_profile(str(output_dir).encode())
            print(f"profile: {n} file(s) written to {output_dir}",
                  file=sys.stderr)

    mod.set_axon_ntff_profile_hook(_hook)


def kernel(x, wq, wk, wv, wo, qk_scale, _profile=False):
    from concourse.bass_utils import run_bass_kernel_spmd

    if _profile:
        _install_profile_hook()

    if "nc" not in _CACHE:
        nc = _build_program()
        nc.finalize()
        _CACHE["nc"] = nc
    nc = _CACHE["nc"]
    in_maps = _host_prep(np.asarray(x), np.asarray(wq), np.asarray(wk),
                         np.asarray(wv), np.asarray(wo), np.asarray(qk_scale))
    res = run_bass_kernel_spmd(nc, in_maps, core_ids=list(range(NCORES)),
                               trace=_profile)
    outs = res.results
    y = np.empty((B, S, DIM), dtype=np.float32)
    for b in range(B):
        y[b] = sum(outs[b * TP + t]["Y"] for t in range(TP))
    if _profile:
        _CACHE["last_exec_time_ns"] = res.exec_time_ns
        _CACHE["last_profile"] = res.profile_json
    return y


# revision 21
# speedup vs baseline: 1.2260x; 1.0054x over previous
"""nn_Attention Trainium2 Bass kernel (optimized).

Full attention forward: x->(q,k,v) with l2-normalized weights, per-head-dim
l2 norm + learned qk scale, interleaved RoPE, causal SDPA, output projection
with column-l2-normalized wo.

Sharding: TP=4 over heads (8 heads/core) x DP=2 over batch across 8 cores.
Each core computes a partial [2048, 2048] output for its batch; host sums
the 4 TP partials per batch.

Key optimizations over the first working version:
- host-side per-partition-contiguous DRAM layouts (64KB descriptors);
  x streamed per 512-row block so the first projection wave starts after
  ~2MB of DMA instead of 16MB.
- projection waves software-pipelined: psum released early via a Pool-engine
  copy; the qT/kT transposes of wave w are emitted after wave w+1's matmuls
  so the PE never waits on the vector norm/rope chain (keeps the PE p-state
  ramped at 2.4 GHz).
- rsqrt via reciprocal_approx_fast (+scalar sqrt) instead of the slow DVE
  reciprocal; rope rotate-half via strided half-views (no rot copies).
- attention: pv psum copied to SBUF by the Pool engine immediately
  (psum ring of 2), denominators batched per block ([8,512] fast recip),
  1/denom broadcast via one packed bf16 matmul per head pair, normalize +
  output projection software-pipelined across blocks (block i+1 attention |
  block i normalize | block i-1 yproj) so the PE stream never breaks.
- mask multiplies, psum->sbuf copies on the otherwise-idle Pool engine.
"""
import sys
import os
import math
from contextlib import ExitStack

sys.path.insert(0, "/opt/trn_rl_repo")

import numpy as np
import ml_dtypes

BF16 = ml_dtypes.bfloat16

B, S, DIM = 2, 2048, 2048
HEADS, DH = 32, 64
THETA = 10000.0
NCORES = 8
TP = 4             # head-parallel ways
HPC = HEADS // TP  # heads per core = 8
E = HPC * DH       # per-core qkv width = 512
ET = E // 128      # e-tiles per core = 4
DT = DIM // 128    # contraction d-tiles = 16
SB = S // 512      # 512-wide seq blocks = 4
SS = S // 128      # 128-wide seq blocks = 16

_CACHE = {}


def _l2n(w, axis):
    n = np.sqrt((w.astype(np.float64) ** 2).sum(axis=axis, keepdims=True))
    n = np.maximum(n, 1e-12)
    return (w / n).astype(np.float32)


def _build_program():
    import concourse.bass as bass
    from concourse import bacc
    import concourse.mybir as mybir
    import concourse.tile as tile
    from concourse.masks import make_identity

    f32 = mybir.dt.float32
    bf16 = mybir.dt.bfloat16
    AF = mybir.ActivationFunctionType
    AX = mybir.AxisListType
    OP = mybir.AluOpType

    nc = bacc.Bacc("TRN2", target_bir_lowering=False)

    xd = nc.dram_tensor("xd", [128, SB * DT * 512], bf16, kind="ExternalInput")
    wqd = nc.dram_tensor("wqd", [128, DT * E], bf16, kind="ExternalInput")
    wkd = nc.dram_tensor("wkd", [128, DT * E], bf16, kind="ExternalInput")
    wvd = nc.dram_tensor("wvd", [128, DT * E], bf16, kind="ExternalInput")
    wod = nc.dram_tensor("wod", [128, ET * DIM], bf16, kind="ExternalInput")
    cosd = nc.dram_tensor("cosd", [128, SS * DH], bf16, kind="ExternalInput")
    sind = nc.dram_tensor("sind", [128, SS * DH], bf16, kind="ExternalInput")
    maskd = nc.dram_tensor("maskd", [128, 4 * 512], bf16, kind="ExternalInput")
    seld = nc.dram_tensor("seld", [33, 128], bf16, kind="ExternalInput")
    Y = nc.dram_tensor("Y", [S, DIM], f32, kind="ExternalOutput")

    with tile.TileContext(nc) as tc, ExitStack() as ctx:
        const = ctx.enter_context(tc.tile_pool(name="const", bufs=1))
        wpool = ctx.enter_context(tc.tile_pool(name="wpool", bufs=1))
        qkv = ctx.enter_context(tc.tile_pool(name="qkv", bufs=1))

        # constants (emitted after the first-wave wq/x0 DMAs below)
        cos_sb = const.tile([128, SS, DH], bf16)
        sin_sb = const.tile([128, SS, DH], bf16)
        mask_sb = const.tile([128, 4, 512], bf16)
        ident = const.tile([128, 128], bf16)
        sel2 = const.tile([33, 128], bf16)

        def load_consts():
            nc.sync.dma_start(cos_sb, cosd.rearrange("p (b d) -> p b d", d=DH))
            nc.sync.dma_start(sin_sb, sind.rearrange("p (b d) -> p b d", d=DH))
            nc.sync.dma_start(mask_sb,
                              maskd.rearrange("p (r n) -> p r n", n=512))
            make_identity(nc, ident)
            nc.sync.dma_start(sel2, seld[:, :])


        # persistent activations
        qT = [qkv.tile([128, S], bf16, tag=f"qT{e}", name=f"qT{e}")
              for e in range(ET)]
        kT = [qkv.tile([128, S], bf16, tag=f"kT{e}", name=f"kT{e}")
              for e in range(ET)]
        v_sb = qkv.tile([128, SS, HPC, 65], bf16, tag="v")
        nc.vector.memset(v_sb[:, :, :, 64:65], 1.0)

        # ---------------- projection phase ----------------
        pools = {}

        def emit_mms(w_sb_, x_t, wname):
            pss = [pools["mm"].tile([128, E], f32, tag="mm",
                                    name=f"ps_{wname}_{su}")
                   for su in range(4)]
            for dt in range(DT):
                for su in range(4):
                    nc.tensor.matmul(
                        pss[su], x_t[:, dt, su * 128:(su + 1) * 128],
                        w_sb_[:, dt, :],
                        start=(dt == 0), stop=(dt == DT - 1))
            return pss

        def emit_chain_qk(ps_s, st, su):
            """norm + rope for one si-128 block; returns bf16 qo tile."""
            sblk = st * 4 + su
            sq = pools["work"].tile([128, E], f32, tag="sq", bufs=2)
            nc.scalar.square(sq, ps_s)
            ssq = pools["work"].tile([128, HPC], f32, tag="ssq", bufs=2)
            nc.vector.tensor_reduce(
                ssq, sq.rearrange("p (h d) -> p h d", d=DH),
                axis=AX.X, op=OP.add)
            rr = pools["work"].tile([128, HPC], f32, tag="rr", bufs=2)
            nc.vector.reciprocal_approx_fast(rr, ssq)
            inv = pools["work"].tile([128, HPC], f32, tag="inv", bufs=2)
            nc.scalar.sqrt(inv, rr)  # 1/|q| per (row, head)
            qn = pools["work"].tile([128, HPC, DH], f32, tag="qn", bufs=2)
            nc.gpsimd.tensor_mul(
                qn, ps_s.rearrange("p (h d) -> p h d", d=DH),
                inv.unsqueeze(2).broadcast_to([128, HPC, DH]))
            # rope: q' = qn*cos + swap32(qn)*sin  (sign folded into sin table)
            cosb = cos_sb[:, sblk:sblk + 1, :].broadcast_to([128, HPC, DH])
            qf = pools["work"].tile([128, HPC, DH], f32, tag="qf", bufs=2)
            nc.vector.tensor_mul(qf, qn, cosb)
            qn4 = qn.rearrange("p h (t u) -> p h t u", u=32)
            sin4 = sin_sb[:, sblk:sblk + 1, :].rearrange(
                "p b (t u) -> p b t u", u=32)
            tmp = pools["work"].tile([128, HPC, 2, 32], f32, tag="tmp", bufs=2)
            nc.vector.tensor_mul(
                tmp[:, :, 0:1, :], qn4[:, :, 1:2, :],
                sin4[:, :, 0:1, :].broadcast_to([128, HPC, 1, 32]))
            nc.vector.tensor_mul(
                tmp[:, :, 1:2, :], qn4[:, :, 0:1, :],
                sin4[:, :, 1:2, :].broadcast_to([128, HPC, 1, 32]))
            qo = pools["work"].tile([128, E], bf16, tag="qo", bufs=3)
            nc.vector.tensor_add(
                qo, qf.rearrange("p h d -> p (h d)"),
                tmp.rearrange("p h t u -> p (h t u)"))
            return qo

        def emit_transposes(dstT, st, qos):
            for su, qo in enumerate(qos):
                sblk = st * 4 + su
                for et in range(ET):
                    trp = pools["tr"].tile([128, 128], bf16, tag="tr",
                                           name=f"tr{sblk}_{et}")
                    nc.tensor.transpose(trp, qo[:, et * 128:(et + 1) * 128],
                                        ident)
                    nc.vector.tensor_copy(
                        dstT[et][:, sblk * 128:(sblk + 1) * 128], trp)

        with tc.tile_pool(name="qkvw", bufs=1) as qkvw, \
             tc.tile_pool(name="xpool", bufs=2) as xpool, \
             tc.tile_pool(name="work", bufs=1) as work, \
             tc.tile_pool(name="pr_mm", bufs=6, space="PSUM") as mm_pool, \
             tc.tile_pool(name="pr_tr", bufs=2, space="PSUM") as tr_pool:
            pools["mm"] = mm_pool
            pools["tr"] = tr_pool
            pools["work"] = work

            # weights: wq first (first wave), wo into the persistent pool
            def load_w(dst, srcd, nchunk=8):
                fl = dst.rearrange("p t e -> p (t e)")
                tot = fl.shape[1]
                ch = tot // nchunk
                for c in range(nchunk):
                    nc.sync.dma_start(fl[:, c * ch:(c + 1) * ch],
                                      srcd[:, c * ch:(c + 1) * ch])

            wq_sb = qkvw.tile([128, DT, E], bf16, tag="wq")
            load_w(wq_sb, wqd)

            def load_x(st):
                xt = xpool.tile([128, DT, 512], bf16, tag="x", name=f"x{st}")
                fl = xt.rearrange("p t s -> p (t s)")
                base = st * DT * 512
                ch = DT * 512 // 16
                for c in range(16):
                    nc.sync.dma_start(fl[:, c * ch:(c + 1) * ch],
                                      xd[:, base + c * ch:base + (c + 1) * ch])
                return xt

            x_cur = load_x(0)
            load_consts()
            wk_sb = qkvw.tile([128, DT, E], bf16, tag="wk")
            wv_sb = qkvw.tile([128, DT, E], bf16, tag="wv")
            wo_sb = wpool.tile([128, ET, DIM], bf16, tag="wo")
            load_w(wk_sb, wkd)
            load_w(wv_sb, wvd)
            load_w(wo_sb, wod)

            pending = None  # (dstT, st, qos) transposes deferred one wave
            for st in range(SB):
                x_nxt = load_x(st + 1) if st + 1 < SB else None
                for wname, w_sb_, dstT in ((f"q{st}", wq_sb, qT),
                                           (f"k{st}", wk_sb, kT),
                                           (f"v{st}", wv_sb, None)):
                    pss = emit_mms(w_sb_, x_cur, wname)
                    if dstT is None:
                        for su in range(4):
                            eng = nc.scalar.copy if su % 2 == 0 \
                                else nc.vector.tensor_copy
                            eng(v_sb[:, st * 4 + su, :, 0:64],
                                pss[su].rearrange("p (h d) -> p h d", d=DH))
                        qos = None
                    else:
                        # release psum banks first (Pool is in-order; the
                        # chains below stall it on ACT results)
                        ps_ss = []
                        for su in range(4):
                            ps_s = pools["work"].tile(
                                [128, E], f32, tag="pss",
                                bufs=4, name=f"pss_{wname}_{su}")
                            if su % 2 == 0:
                                nc.scalar.copy(ps_s, pss[su])
                            else:
                                nc.vector.tensor_copy(ps_s, pss[su])
                            ps_ss.append(ps_s)
                        qos = [emit_chain_qk(ps_ss[su], st, su)
                               for su in range(4)]
                    if pending is not None:
                        emit_transposes(*pending)
                        pending = None
                    if qos is not None:
                        pending = (dstT, st, qos)
                x_cur = x_nxt
            if pending is not None:
                emit_transposes(*pending)

        # ---------------- attention + output projection ----------------
        def emit_lg(i, h, p, pvh):
            """logit pair matmuls for (head h, si-block i, pair p)."""
            et, hp = h // 2, (h % 2) * 64
            lg2 = pools["lg"].tile([128, 2, 512], f32, tag="lg",
                                   name=f"lg{i}_{h}_{p}")
            for bb in range(2):
                sjb = 2 * p + bb
                nc.tensor.matmul(
                    lg2[:, bb, :],
                    kT[et][hp:hp + 64, sjb * 128:(sjb + 1) * 128],
                    qT[et][hp:hp + 64, i * 512:(i + 1) * 512],
                    start=True, stop=True)
            return lg2

        def pair_step(i, h, p, lg2, pvh):
            """exp + mask + pv accumulation for one (h, pair) step."""
            ex = pools["attnp"].tile([128, 2, 512], bf16, tag="ex", bufs=4)
            r0 = 2 * p - 4 * i  # diag offset of the pair's first block
            c0p = max(0, r0) * 128
            if c0p > 0:
                nc.scalar.activation(ex[:, :, c0p:], lg2[:, :, c0p:], AF.Exp)
            else:
                nc.scalar.activation(ex, lg2, AF.Exp)
            return ex

        def pair_pv(i, h, p, ex, pvh):
            for bb in range(2):
                sjb = 2 * p + bb
                r = sjb - 4 * i
                r0 = 2 * p - 4 * i
                c0p = max(0, r0) * 128
                if r >= 0:
                    ce = (r + 1) * 128
                    nc.gpsimd.tensor_mul(ex[:, bb, c0p:ce],
                                         ex[:, bb, c0p:ce],
                                         mask_sb[:, r, c0p:ce])
                cpv = max(0, r) * 128
                nc.tensor.matmul(
                    pvh[0:65, cpv:],
                    v_sb[:, sjb, h, :],
                    ex[:, bb, cpv:],
                    start=(sjb == 0), stop=(sjb == 4 * i + 3),
                    skip_group_check=True)

        def normalize_pair(pvs_list, inv8b, i, j):
            """divide heads 2j,2j+1 of block i by softmax denominators."""
            bc = pools["yb"].tile([128, 512], f32, tag="yb",
                                  name=f"bc{i}_{j}")
            nc.tensor.matmul(bc, sel2, inv8b[:, j, :],
                             start=True, stop=True)
            for kk in range(2):
                h = 2 * j + kk
                et, hp = h // 2, (h % 2) * 64
                nc.vector.tensor_mul(
                    qT[et][hp:hp + 64, i * 512:(i + 1) * 512],
                    pvs_list[h][0:64, :], bc[kk * 64:(kk + 1) * 64, :])

        def yproj(ib):
            """si-128 block ib -> Y[ib*128:+128, :]."""
            for nd in range(4):
                ps = pools["yb"].tile([128, 512], f32, tag="yb",
                                      name=f"yps{ib}_{nd}")
                for ket in range(ET):
                    nc.tensor.matmul(
                        ps,
                        qT[ket][:, ib * 128:(ib + 1) * 128],
                        wo_sb[:, ket, nd * 512:(nd + 1) * 512],
                        start=(ket == 0), stop=(ket == ET - 1))
                ys = pools["ypool"].tile([128, 512], f32, tag="y", bufs=4)
                if nd % 2 == 0:
                    nc.scalar.copy(ys, ps)
                else:
                    nc.vector.tensor_copy(ys, ps)
                nc.sync.dma_start(
                    Y[ib * 128:(ib + 1) * 128, nd * 512:(nd + 1) * 512], ys)

        with tc.tile_pool(name="attnp", bufs=1) as attnp, \
             tc.tile_pool(name="ypool", bufs=1) as ypool, \
             tc.tile_pool(name="ps_lg", bufs=2, space="PSUM") as lg_pool, \
             tc.tile_pool(name="ps_pv", bufs=2, space="PSUM") as pv_pool, \
             tc.tile_pool(name="ps_yb", bufs=2, space="PSUM") as yb_pool:
            pools["lg"] = lg_pool
            pools["pv"] = pv_pool
            pools["yb"] = yb_pool
            pools["attnp"] = attnp
            pools["ypool"] = ypool
            prev = None  # (i, pvs_list, inv8b) of previous block
            dn8 = pools["attnp"].tile([33, 4, 512], f32, tag="dn8", bufs=1,
                                      name="dn8")
            nc.vector.memset(dn8, 1.0)  # rows 1..31 stay 1.0 (recip-safe)
            for i in range(SB):
                npr = 2 * (i + 1)
                steps = [(h, p) for h in range(HPC) for p in range(npr)]
                pvt = {}
                for h in range(HPC):
                    pvt[h] = None  # allocated lazily at first lg emission
                lgt = {}
                exd = {}

                def get_pv(h):
                    if pvt[h] is None:
                        pvt[h] = pv_pool.tile([128, 512], f32, tag="pv",
                                              name=f"pv{i}_{h}")
                    return pvt[h]

                # prime lookahead of 2 pair-steps
                for k in range(min(2, len(steps))):
                    h, p = steps[k]
                    lgt[(h, p)] = emit_lg(i, h, p, get_pv(h))
                cur_pvs = []
                for k, (h, p) in enumerate(steps):
                    lg2 = lgt.pop((h, p))
                    ex = pair_step(i, h, p, lg2, get_pv(h))
                    if k + 2 < len(steps):
                        h2, p2 = steps[k + 2]
                        lgt[(h2, p2)] = emit_lg(i, h2, p2, get_pv(h2))
                    pair_pv(i, h, p, ex, get_pv(h))
                    if p == npr - 1:
                        # head h complete: drain pv psum, denominators
                        pv = pvt[h]
                        pvs = pools["attnp"].tile([65, 512], f32, tag="pvs",
                                                  bufs=12, name=f"pvs{i}_{h}")
                        nc.vector.tensor_copy(pvs, pv[0:65, :])  # frees psum
                        pr = (h % 2) * 32
                        nc.scalar.copy(dn8[pr:pr + 1, h // 2, :],
                                       pv[64:65, :])
                        cur_pvs.append(pvs)
                        if h % 2 == 1 and prev is not None:
                            j = h // 2
                            pi, ppvs, pinv = prev
                            normalize_pair(ppvs, pinv, pi, j)
                            if pi > 0:
                                yproj(4 * (pi - 1) + j)
                inv8 = pools["attnp"].tile([33, 4, 512], f32, tag="inv8",
                                           bufs=1, name=f"inv8_{i}")
                nc.vector.reciprocal_approx_fast(inv8, dn8)
                inv8b = pools["attnp"].tile([33, 4, 512], bf16, tag="inv8b",
                                            bufs=2, name=f"inv8b_{i}")
                nc.scalar.copy(inv8b, inv8)
                prev = (i, cur_pvs, inv8b)
            # flush: yproj block 2 first (fills PE while the last
            # denominators come through recip/cast), then block 3
            pi, ppvs, pinv = prev
            for j in range(4):
                yproj(4 * (pi - 1) + j)
                normalize_pair(ppvs, pinv, pi, j)
            for ib in range(4 * pi, 4 * pi + 4):
                yproj(ib)

    return nc


def _host_prep(x, wq, wk, wv, wo, qk_scale):
    """Returns per-core input dicts."""
    perm = np.concatenate([np.arange(0, DH, 2), np.arange(1, DH, 2)])
    wq_n = _l2n(wq, -1).reshape(HEADS, DH, DIM)[:, perm, :].reshape(HEADS * DH, DIM)
    wk_n = _l2n(wk, -1).reshape(HEADS, DH, DIM)[:, perm, :].reshape(HEADS * DH, DIM)
    wv_n = _l2n(wv, -1)
    wo_n = _l2n(wo, 0)
    sp = qk_scale.astype(np.float64)[perm]

    # rope tables with qk_scale folded in; permuted-block layout
    half = np.arange(0, DH, 2)
    freqs = 1.0 / (THETA ** (half.astype(np.float64) / DH))      # (32,)
    ang = np.arange(S, dtype=np.float64)[:, None] * freqs[None]  # (S, 32)
    cos_h, sin_h = np.cos(ang), np.sin(ang)
    cos_p = np.concatenate([cos_h, cos_h], 1)                    # (S, 64)
    sin_e = np.concatenate([-sin_h, sin_h], 1)
    cos_eff = (cos_p * sp[None, :]).astype(np.float32)
    swap_sp = np.concatenate([sp[32:], sp[:32]])
    sin_eff = (sin_e * swap_sp[None, :]).astype(np.float32)
    # device layout [128, SS*DH]: [p, b*64+c] = tbl[b*128+p, c]
    cosd = np.ascontiguousarray(
        cos_eff.reshape(SS, 128, DH).transpose(1, 0, 2).reshape(128, SS * DH))
    sind = np.ascontiguousarray(
        sin_eff.reshape(SS, 128, DH).transpose(1, 0, 2).reshape(128, SS * DH))

    # causal masks for the 4 diagonal offsets: keep sjl + 128r <= sil
    sjl = np.arange(128)[:, None]
    sil = np.arange(512)[None, :]
    maskd = np.ascontiguousarray(np.concatenate(
        [(sjl + 128 * r <= sil).astype(np.float32) for r in range(4)],
        axis=1))  # [128, 4*512]

    def wlay(a):  # [E, DIM] -> [128, DT*E] with [p, t*E+e] = a[e, t*128+p]
        return np.ascontiguousarray(
            a.T.reshape(DT, 128, E).transpose(1, 0, 2).reshape(128, DT * E))

    sel2h = np.zeros((33, 128), dtype=np.float32)
    sel2h[0, 0:64] = 1.0
    sel2h[32, 64:128] = 1.0

    in_maps = []
    for c in range(NCORES):
        b, t = divmod(c, TP)
        e0 = t * E
        xb = x[b]  # [S, DIM]
        xdev = np.ascontiguousarray(
            xb.reshape(SB, 512, DT, 128).transpose(3, 0, 2, 1)
            .reshape(128, SB * DT * 512))
        wob = wo_n[:, e0:e0 + E]  # [DIM, E]
        wodev = np.ascontiguousarray(
            wob.T.reshape(ET, 128, DIM).transpose(1, 0, 2)
            .reshape(128, ET * DIM))
        in_maps.append({
            "xd": xdev.astype(BF16),
            "wqd": wlay(wq_n[e0:e0 + E]).astype(BF16),
            "wkd": wlay(wk_n[e0:e0 + E]).astype(BF16),
            "wvd": wlay(wv_n[e0:e0 + E]).astype(BF16),
            "wod": wodev.astype(BF16),
            "cosd": cosd.astype(BF16), "sind": sind.astype(BF16),
            "seld": sel2h.astype(BF16),
            "maskd": maskd.astype(BF16),
        })
    return in_maps


def _install_profile_hook():
    """antenv.axon_hooks is absent in this image; shim it and register the
    ctypes NTFF hook against /opt/axon/libaxon_pjrt.so (mirrors trn_boot)."""
    import types
    import ctypes
    import contextlib

    try:
        from antenv.axon_hooks import get_axon_ntff_profile_hook  # noqa
        return
    except ImportError:
        pass
    import antenv
    mod = types.ModuleType("antenv.axon_hooks")
    state = {}
    mod.set_axon_ntff_profile_hook = lambda h: state.__setitem__("h", h)
    mod.get_axon_ntff_profile_hook = lambda: state.get("h")
    sys.modules["antenv.axon_hooks"] = mod
    antenv.axon_hooks = mod

    so_path = "/opt/axon/libaxon_pjrt.so"
    lib = ctypes.CDLL(so_path)
    if not hasattr(lib, "axon_start_nrt_profile"):
        return
    lib.axon_start_nrt_profile.argtypes = [
        ctypes.POINTER(ctypes.c_int64), ctypes.c_size_t]
    lib.axon_start_nrt_profile.restype = ctypes.c_int64
    lib.axon_stop_nrt_profile.argtypes = [ctypes.c_char_p]
    lib.axon_stop_nrt_profile.restype = ctypes.c_int64

    @contextlib.contextmanager
    def _hook(output_dir, device_ids):
        import jax
        jax.devices()
        if device_ids:
            ids = (ctypes.c_int64 * len(device_ids))(*device_ids)
            rc = lib.axon_start_nrt_profile(ids, len(device_ids))
        else:
            rc = lib.axon_start_nrt_profile(None, 0)
        if rc != 0:
            raise RuntimeError(f"axon_start_nrt_profile rc={rc}")
        try:
            yield
        finally:
            n = lib.axon_stop_nrt_profile(str(output_dir).encode())
            print(f"profile: {n} file(s) written to {output_dir}",
                  file=sys.stderr)

    mod.set_axon_ntff_profile_hook(_hook)


def kernel(x, wq, wk, wv, wo, qk_scale, _profile=False):
    from concourse.bass_utils import run_bass_kernel_spmd

    if _profile:
        _install_profile_hook()

    if "nc" not in _CACHE:
        nc = _build_program()
        nc.finalize()
        _CACHE["nc"] = nc
    nc = _CACHE["nc"]
    in_maps = _host_prep(np.asarray(x), np.asarray(wq), np.asarray(wk),
                         np.asarray(wv), np.asarray(wo), np.asarray(qk_scale))
    res = run_bass_kernel_spmd(nc, in_maps, core_ids=list(range(NCORES)),
                               trace=_profile)
    outs = res.results
    y = np.empty((B, S, DIM), dtype=np.float32)
    for b in range(B):
        y[b] = sum(outs[b * TP + t]["Y"] for t in range(TP))
    if _profile:
        _CACHE["last_exec_time_ns"] = res.exec_time_ns
        _CACHE["last_profile"] = res.profile_json
    return y
